# revision 1
# baseline (speedup 1.0000x reference)
"""DTW loss (0.5*MAE + 0.5*DTW(pred[0],target[0])/(S*F)) on 8 TRN2 cores.

v3: slope-3 anti-diagonal wavefront with a fused (min,+) scan.

The [S,S] DP is split into NB=128 column chunks of W=16, chunk c in SBUF
partition c.  At wavefront step t partition c processes DP row i = t - 3*c.
The whole row recurrence D[i,j] = d[i,j] + min(D[i-1,j-1], D[i-1,j], D[i,j-1])
is ONE hardware TensorTensorScan per step:

    state = seed;  state = (m[j] min state) add d[j]

with m[j] = min(D[i-1,j-1], D[i-1,j]) (one pair-min tensor_tensor) and
seed = D[i, 16c-1] from the left-neighbour chunk via a tiny PE shift-matmul
(batched two steps per matmul, 3 steps of slack so PE stays off the critical
path).  So the serial chain is just 2 DVE instructions per step.

Storage is a 17-column-per-step in-place ring: block(t) = dsk[:, 17*(t+3)+0:17]
holds [halo, d0..d15] and the scan overwrites it with [halo, D0..D15]
(out == data1; col0 has d=0 so out[0] = seed, materializing next step's halo).
Out-of-range regions are poisoned BIG so no masking is needed.

d[i,j] = sqrt(|x_i|^2+|y_j|^2-2 x.y) is produced on the fly: bf16 GEMM with
the squared norms folded in as a rank-2 augmented matmul -> ACT sqrt (written
with a 17-stride gapped AP) -> block-fused skew DMAs, all paced into the
wavefront's idle engine time.  x^T/y^T/norms are precomputed host-side.

The MAE term is data-parallel over the batch; each core reduces its 2-sample
shard on DVE in 8 chunks interleaved into the wavefront (DMA-prefetched).
"""

import numpy as np

_BIG = 1.0e30
_S, _F, _B = 2048, 128, 16
_W = 16
_BW = 17
_SLOPE = 3
_SCAN_REPS = 1  # >1 only for timing experiments (corrupts the DP)


# ---------------------------------------------------------------- patches
def _apply_walrus_patches():
    """This walrus build rejects >1 semaphore wait per instruction; peel
    extra waits onto same-engine Drain instructions."""
    import bass_rust
    import concourse.mybir as mybir
    from concourse import tile
    from concourse.vector_clock import ScopedClock

    if getattr(tile.TileContext, "_wait_split_patched", False):
        return
    _orig_add = tile.TileContext._add_instruction

    def _mk_drain(nc, engine, waits):
        d = mybir.InstDrain(name=nc.get_next_instruction_name(), engine=engine)
        d.sync_info = bass_rust.SyncInfo(on_wait=list(waits), on_update=[])
        return d

    def _add_split(self, inst):
        si = inst.sync_info
        if (
            si is not None
            and si.on_wait
            and len(si.on_wait) > 1
            and inst.engine is not None
            and inst.engine != mybir.EngineType.Unassigned
        ):
            waits = list(si.on_wait)
            si.on_wait = waits[:1]
            for w in waits[1:]:
                _orig_add(self, _mk_drain(self.nc, inst.engine, [w]))
        _orig_add(self, inst)

    def _drain_and_barrier_split(self, tick_clock, wait_clock):
        nc = self.nc
        drain_inst = nc.sync.drain()
        wait_clock.add_sem_waits(
            drain_inst.ins, ScopedClock({None: tick_clock.global_clock})
        )
        si = drain_inst.ins.sync_info
        waits = list(si.on_wait) if si and si.on_wait else []
        if len(waits) > 1:
            si.on_wait = waits[:1]
            for w in waits[1:]:
                d2 = nc.sync.drain()
                d2.ins.sync_info = bass_rust.SyncInfo(on_wait=[w], on_update=[])
        nc.all_engine_barrier()
        assert self.sems is not None
        popped = nc._tile_sem_poison_stack.pop()
        assert popped is self._sem_poison
        nc.clear_and_free_semaphores(list(self.sems.allocated().values()))
        nc.all_engine_barrier()

    tile.TileContext._add_instruction = _add_split
    tile.TileContext._drain_and_barrier = _drain_and_barrier_split
    tile.TileContext._wait_split_patched = True


# ---------------------------------------------------------------- builder
def _build_nc(S, F, mae_rows):
    import concourse.bass as bass
    import concourse.mybir as mybir
    from concourse import tile

    _apply_walrus_patches()

    f32 = mybir.dt.float32
    bf16 = mybir.dt.bfloat16
    AL = mybir.AluOpType
    AF = mybir.ActivationFunctionType

    W, BW, SL = _W, _BW, _SLOPE
    NB = S // W                          # 128 column chunks
    NSTEP = S + SL * (NB - 1)            # 2429 wavefront steps
    NBLK = NSTEP + SL                    # absolute 17-col blocks
    DSKC = BW * NBLK                     # dsk free cols (41344)
    HEADC = BW * (SL * NB)               # head poison cols [0, 6528)
    TAILC = BW * (SL + S)                # tail poison from col 34867

    nc = bass.Bass("TRN2", target_bir_lowering=False, debug=False, num_devices=8)

    xtn_d = nc.dram_tensor("xtn", [F, S], bf16, kind="ExternalInput").ap()
    yt_d = nc.dram_tensor("yt", [F, S], bf16, kind="ExternalInput").ap()
    xq_d = nc.dram_tensor("xq", [2, S], bf16, kind="ExternalInput").ap()
    yq_d = nc.dram_tensor("yq", [2, S], bf16, kind="ExternalInput").ap()
    mp_d = nc.dram_tensor("mp", [mae_rows, F], f32, kind="ExternalInput").ap()
    mt_d = nc.dram_tensor("mt", [mae_rows, F], f32, kind="ExternalInput").ap()
    shiftT_d = nc.dram_tensor("shiftT", [128, 128], f32, kind="ExternalInput").ap()
    bigrow_d = nc.dram_tensor("bigrow", [1, 128], f32, kind="ExternalInput").ap()
    onetwo_d = nc.dram_tensor("onetwo", [1, 2], f32, kind="ExternalInput").ap()
    dtw_d = nc.dram_tensor("dtw", [1, 1], f32, kind="ExternalOutput").ap()
    mae_d = nc.dram_tensor("mae", [1, 1], f32, kind="ExternalOutput").ap()

    mae_free = mae_rows * F // 128       # 4096 cols per partition
    MCH = 512
    n_mch = mae_free // MCH              # 8 chunks

    with tile.TileContext(nc) as tc:
        with (
            tc.tile_pool(name="big", bufs=1) as bp,
            tc.tile_pool(name="stg", bufs=2) as stgp,
            tc.tile_pool(name="mch", bufs=4) as mchp,
            tc.tile_pool(name="red", bufs=2) as redp,
            tc.tile_pool(name="gps", bufs=2, space=bass.MemorySpace.PSUM) as gpsp,
            tc.tile_pool(name="sps", bufs=4, space=bass.MemorySpace.PSUM) as spsp,
        ):
            dsk = bp.tile([128, DSKC], f32, tag="dsk")
            XTN = bp.tile([128, S], bf16, tag="XTN")
            YT = bp.tile([128, S], bf16, tag="YT")
            xq = bp.tile([2, S], bf16, tag="xq")
            yq = bp.tile([2, S], bf16, tag="yq")
            mB = bp.tile([128, BW], f32, tag="mB")
            shT = bp.tile([128, 128], f32, tag="shT")
            brow = bp.tile([1, 128], f32, tag="brow")
            otwo = bp.tile([1, 2], f32, tag="otwo")
            ones128 = bp.tile([128, 1], f32, tag="ones128")
            macc = bp.tile([128, 1], f32, tag="macc")
            mres = bp.tile([1, 1], f32, tag="mres")

            # ---------------- prologue: loads + poison ------------------
            nc.sync.dma_start(XTN[:], xtn_d)
            nc.sync.dma_start(YT[:], yt_d)
            nc.sync.dma_start(xq[:], xq_d)
            nc.sync.dma_start(yq[:], yq_d)
            nc.sync.dma_start(shT[:], shiftT_d)
            nc.sync.dma_start(brow[:], bigrow_d)
            nc.sync.dma_start(otwo[:], onetwo_d)
            nc.gpsimd.memset(ones128[:], 1.0)
            nc.gpsimd.memset(macc[:], 0.0)
            nc.gpsimd.memset(mB[:, 0:1], _BIG)
            nc.vector.memset(dsk[:, 0:HEADC], _BIG)
            nc.gpsimd.memset(dsk[:, TAILC:DSKC], _BIG)
            # DP start cell: block(-1)[chunk0, col0] = 0 enables D[0,0]=d[0,0]
            nc.gpsimd.memset(dsk[0:1, (SL - 1) * BW:(SL - 1) * BW + 1], 0.0)

            # zero col0 of all 32 chunk-groups in both stg buffers (the skew
            # DMA carries them into block col0 = the scan's "d=0" halo slot)
            stg_tiles = []
            for _ in range(2):
                s = stgp.tile([128, 32 * BW], f32, tag="stg")
                for cl in range(32):
                    nc.gpsimd.memset(s[:, cl * BW:cl * BW + 1], 0.0)
                stg_tiles.append(s)

            # ---------------- d production ------------------------------
            # block (ib, jc): DP rows [128ib, 128ib+128) x cols [512jc, +512)
            def emit_mm(pg, ib, jc, sl):
                i0, a = 128 * ib, 512 * jc + 128 * sl
                nc.tensor.matmul(
                    pg[:, 128 * sl:128 * sl + 128],
                    XTN[:, i0:i0 + 128], YT[:, a:a + 128],
                    start=True, stop=False,
                )
                nc.tensor.matmul(
                    pg[:, 128 * sl:128 * sl + 128],
                    xq[:, i0:i0 + 128], yq[:, a:a + 128],
                    start=False, stop=True,
                )

            def emit_sqrt(pg, st, sl):
                gap = bass.AP(
                    st.tensor, BW * 8 * sl + 1,
                    [[32 * BW, 128], [BW, 8], [1, W]],
                )
                nc.scalar.activation(gap, pg[:, 128 * sl:128 * sl + 128], AF.Sqrt)

            def emit_dma(st, ib, jc, h):
                # 8 per-chunk DMAs (BIR APs cannot step partitions off-pitch);
                # issued from the Pool queue whose DMA dispatch is ~20x
                # cheaper than SP's
                i0 = 128 * ib
                for cl in range(8 * h, 8 * h + 8):
                    c = 32 * jc + cl
                    src = st[:, cl * BW:(cl + 1) * BW]
                    dst = bass.AP(
                        dsk.tensor,
                        c * DSKC + (i0 + SL * c + SL) * BW,
                        [[DSKC, 1], [BW, 128], [1, BW]],
                    )
                    nc.gpsimd.dma_start(dst, src)

            def emit_block(ib, jc):
                pg = gpsp.tile([128, 512], f32, tag="pg")
                st = stg_tiles[0]
                stg_tiles.reverse()
                for sl in range(4):
                    emit_mm(pg, ib, jc, sl)
                for sl in range(4):
                    emit_sqrt(pg, st, sl)
                for h in range(4):
                    emit_dma(st, ib, jc, h)

            # first 3 i-blocks fully in the prologue (needed from step 0)
            for ib in range(3):
                for jc in range(4):
                    emit_block(ib, jc)

            # remaining 52 blocks paced into the wavefront, deadline order.
            # each block = 10 slots (4 mm-pairs, 4 sqrts, 2 dma) 3 steps apart
            rest = sorted(
                [(ib, jc) for ib in range(3, S // 128) for jc in range(4)],
                key=lambda b: 128 * b[0] + 96 * b[1],
            )
            sched = {}
            blk_state = {}
            for n, (ib, jc) in enumerate(rest):
                base = 10 + 32 * n
                for s in range(4):
                    sched.setdefault(base + 3 * s, []).append(("mm", ib, jc, s))
                for s in range(4):
                    sched.setdefault(base + 12 + 3 * s, []).append(("sq", ib, jc, s))
                for h in range(4):
                    sched.setdefault(base + 22 + 2 * h, []).append(("dm", ib, jc, h))

            # MAE chunk k: prefetch DMA at E_k, DVE consume at E_k + 220
            mpv = mp_d.rearrange("(p x) f -> p (x f)", p=128)
            mtv = mt_d.rearrange("(p x) f -> p (x f)", p=128)
            mae_tiles = {}
            for k in range(n_mch):
                sched.setdefault(120 + 260 * k, []).append(("ml", k))
                sched.setdefault(340 + 260 * k, []).append(("mc", k))

            def emit_sched(t):
                for op in sched.pop(t, ()):
                    if op[0] == "mm":
                        _, ib, jc, sl = op
                        if sl == 0:
                            pg = gpsp.tile([128, 512], f32, tag="pg")
                            st = stg_tiles[0]
                            stg_tiles.reverse()
                            blk_state[(ib, jc)] = (pg, st)
                        pg, st = blk_state[(ib, jc)]
                        emit_mm(pg, ib, jc, sl)
                    elif op[0] == "sq":
                        _, ib, jc, sl = op
                        pg, st = blk_state[(ib, jc)]
                        emit_sqrt(pg, st, sl)
                    elif op[0] == "dm":
                        _, ib, jc, h = op
                        pg, st = blk_state[(ib, jc)]
                        emit_dma(st, ib, jc, h)
                    elif op[0] == "ml":
                        k = op[1]
                        ta = mchp.tile([128, MCH], f32, tag="ma")
                        tb = mchp.tile([128, MCH], f32, tag="mb")
                        nc.sync.dma_start(ta[:], mpv[:, k * MCH:(k + 1) * MCH])
                        nc.sync.dma_start(tb[:], mtv[:, k * MCH:(k + 1) * MCH])
                        mae_tiles[k] = (ta, tb)
                    else:  # "mc"
                        k = op[1]
                        ta, tb = mae_tiles.pop(k)
                        nc.vector.tensor_tensor(ta[:], ta[:], tb[:], AL.subtract)
                        red = redp.tile([128, 1], f32, tag="red")
                        nc.vector.tensor_reduce(
                            red[:], ta[:], mybir.AxisListType.X, AL.add,
                            apply_absolute_value=True,
                        )
                        nc.vector.tensor_tensor(macc[:], macc[:], red[:], AL.add)

            # ---------------- wavefront ---------------------------------
            cur_ps = None
            for t in range(NSTEP):
                if t % 2 == 0:
                    # seeds for steps t, t+1: pst[c] = BIG@c0 + D-col16 of
                    # blocks (t-3),(t-2) shifted down one partition
                    cur_ps = spsp.tile([128, 2], f32, tag="pst")
                    nc.tensor.matmul(
                        cur_ps[:, 0:2], brow[:, 0:128], otwo[:],
                        start=True, stop=False,
                    )
                    nc.tensor.matmul(
                        cur_ps[:, 0:2], shT[:, 0:128],
                        dsk[:, BW * t + W:BW * t + W + 2 * BW:BW],
                        start=False, stop=True,
                    )
                emit_sched(t)
                o = BW * (t + SL)
                prev = dsk[:, o - BW:o]
                nc.vector.tensor_tensor(
                    mB[:, 1:BW], prev[:, 0:W], prev[:, 1:BW], AL.min
                )
                for _ in range(_SCAN_REPS):
                    nc.vector.tensor_tensor_scan(
                        dsk[:, o:o + BW], mB[:, 0:BW], dsk[:, o:o + BW],
                        cur_ps[:, (t % 2):(t % 2) + 1], AL.min, AL.add,
                    )
            # leftover sched events (none expected, but don't drop any)
            for t in sorted(list(sched.keys())):
                emit_sched(t)

            # ---------------- outputs -----------------------------------
            nc.sync.dma_start(dtw_d, dsk[NB - 1:NB, DSKC - 1:DSKC])
            pm = gpsp.tile([128, 512], f32, tag="pg")
            nc.tensor.matmul(
                pm[0:1, 0:1], macc[:, 0:1], ones128[:, 0:1],
                start=True, stop=True,
            )
            nc.scalar.copy(mres[0:1, 0:1], pm[0:1, 0:1])
            nc.sync.dma_start(mae_d, mres[0:1, 0:1])

    return nc


# ---------------------------------------------------------------- runtime
_CACHE = {}


def _get_nc(S, F, mae_rows):
    key = (S, F, mae_rows)
    if key not in _CACHE:
        _CACHE[key] = _build_nc(S, F, mae_rows)
    return _CACHE[key]


def _prepare(pred, target):
    import ml_dtypes

    B, S, F = pred.shape
    n_cores = 8
    shard = B // n_cores
    mae_rows = shard * S
    nc = _get_nc(S, F, mae_rows)

    pred = np.ascontiguousarray(pred, dtype=np.float32)
    target = np.ascontiguousarray(target, dtype=np.float32)
    x0, y0 = pred[0], target[0]

    bf = ml_dtypes.bfloat16
    xtn = np.ascontiguousarray((-2.0 * x0.T).astype(bf))        # [F, S]
    yt = np.ascontiguousarray(y0.T.astype(bf))                  # [F, S]
    xsq = (x0 * x0).sum(axis=1, dtype=np.float32)               # [S]
    ysq = (y0 * y0).sum(axis=1, dtype=np.float32)
    xq = np.ascontiguousarray(
        np.stack([xsq, np.ones_like(xsq)]).astype(bf))          # [2, S]
    yq = np.ascontiguousarray(
        np.stack([np.ones_like(ysq), ysq]).astype(bf))          # [2, S]

    shiftT = np.zeros((128, 128), np.float32)
    for c in range(1, 128):
        shiftT[c - 1, c] = 1.0          # pst[c] = col16[c-1]
    bigrow = np.zeros((1, 128), np.float32)
    bigrow[0, 0] = _BIG
    onetwo = np.ones((1, 2), np.float32)

    in_maps = []
    for k in range(n_cores):
        in_maps.append({
            "xtn": xtn, "yt": yt, "xq": xq, "yq": yq,
            "mp": pred[k * shard:(k + 1) * shard].reshape(mae_rows, F),
            "mt": target[k * shard:(k + 1) * shard].reshape(mae_rows, F),
            "shiftT": shiftT, "bigrow": bigrow, "onetwo": onetwo,
        })
    return nc, in_maps


def kernel(pred, target):
    from concourse.bass_utils import run_bass_kernel_spmd

    B, S, F = pred.shape
    nc, in_maps = _prepare(np.asarray(pred), np.asarray(target))
    res = run_bass_kernel_spmd(nc, in_maps, core_ids=list(range(8)))
    mae_sum = sum(float(res.results[k]["mae"][0, 0]) for k in range(8))
    dtw = float(res.results[0]["dtw"][0, 0])
    loss = 0.5 * (mae_sum / (B * S * F)) + 0.5 * (dtw / (S * F))
    return np.float32(loss)



# revision 9
# speedup vs baseline: 3.8438x; 3.8438x over previous
"""DTW loss (0.5*MAE + 0.5*DTW(pred[0],target[0])/(S*F)) on 8 TRN2 cores.

v4: same slope-3 anti-diagonal wavefront DP as v3, but the host->device
contract is slimmed to the information-theoretic minimum and the runtime
path is cached:

* Only two inputs are shipped per core: the bf16 batch shards mp/mt
  ([4096,128] each, 2MB/core).  Everything the DTW needs -- the F-major
  transposes XTN=-2*x^T / YT=y^T, the squared-norm rows xq/yq, the
  shift/identity matrices, poison rows -- is derived ON DEVICE in the
  prologue (PE transposes via identity matmul, ACT Square + ones-matmul
  for the norms, diagonal DMAs for I / shift).  This cuts the axon
  host->device transfer from 42.6MB to 16.8MB (~50MB/s tunnel).

* run_bass_kernel_spmd re-traces + re-lowers a fresh jax.jit on every
  call (~1s of host overhead).  kernel() uses it once (first call:
  compile + validate), then switches to a cached compiled executable of
  the identical shard_map computation, so steady-state calls pay only
  input transfer + execution.

The [S,S] DP is split into NB=128 column chunks of W=16, chunk c in SBUF
partition c.  At wavefront step t partition c processes DP row i = t - 3*c.
The whole row recurrence D[i,j] = d[i,j] + min(D[i-1,j-1], D[i-1,j], D[i,j-1])
is ONE hardware TensorTensorScan per step:

    state = seed;  state = (m[j] min state) add d[j]

with m[j] = min(D[i-1,j-1], D[i-1,j]) (one pair-min tensor_tensor) and
seed = D[i, 16c-1] from the left-neighbour chunk via a tiny PE shift-matmul
(batched two steps per matmul, 3 steps of slack so PE stays off the critical
path).  So the serial chain is just 2 DVE instructions per step.

Storage is a 17-column-per-step in-place ring: block(t) = dsk[:, 17*(t+3)+0:17]
holds [halo, d0..d15] and the scan overwrites it with [halo, D0..D15]
(out == data1; col0 has d=0 so out[0] = seed, materializing next step's halo).
Out-of-range regions are poisoned BIG so no masking is needed.

d[i,j] = sqrt(|x_i|^2+|y_j|^2-2 x.y) is produced on the fly: bf16 GEMM with
the squared norms folded in as a rank-2 augmented matmul -> ACT sqrt (written
with a 17-stride gapped AP) -> block-fused skew DMAs, all paced into the
wavefront's idle engine time.

The MAE term is data-parallel over the batch; each core reduces its 2-sample
shard on DVE in 8 chunks interleaved into the wavefront (DMA-prefetched).
"""

import numpy as np

_BIG = 1.0e30
_S, _F, _B = 2048, 128, 16
_W = 16
_BW = 17
_SLOPE = 3
_SCAN_REPS = 1  # >1 only for timing experiments (corrupts the DP)


# ---------------------------------------------------------------- patches
def _apply_walrus_patches():
    """This walrus build rejects >1 semaphore wait per instruction; peel
    extra waits onto same-engine Drain instructions."""
    import bass_rust
    import concourse.mybir as mybir
    from concourse import tile
    from concourse.vector_clock import ScopedClock

    if getattr(tile.TileContext, "_wait_split_patched", False):
        return
    _orig_add = tile.TileContext._add_instruction

    def _mk_drain(nc, engine, waits):
        d = mybir.InstDrain(name=nc.get_next_instruction_name(), engine=engine)
        d.sync_info = bass_rust.SyncInfo(on_wait=list(waits), on_update=[])
        return d

    def _add_split(self, inst):
        si = inst.sync_info
        if (
            si is not None
            and si.on_wait
            and len(si.on_wait) > 1
            and inst.engine is not None
            and inst.engine != mybir.EngineType.Unassigned
        ):
            waits = list(si.on_wait)
            si.on_wait = waits[:1]
            for w in waits[1:]:
                _orig_add(self, _mk_drain(self.nc, inst.engine, [w]))
        _orig_add(self, inst)

    def _drain_and_barrier_split(self, tick_clock, wait_clock):
        nc = self.nc
        drain_inst = nc.sync.drain()
        wait_clock.add_sem_waits(
            drain_inst.ins, ScopedClock({None: tick_clock.global_clock})
        )
        si = drain_inst.ins.sync_info
        waits = list(si.on_wait) if si and si.on_wait else []
        if len(waits) > 1:
            si.on_wait = waits[:1]
            for w in waits[1:]:
                d2 = nc.sync.drain()
                d2.ins.sync_info = bass_rust.SyncInfo(on_wait=[w], on_update=[])
        nc.all_engine_barrier()
        assert self.sems is not None
        popped = nc._tile_sem_poison_stack.pop()
        assert popped is self._sem_poison
        nc.clear_and_free_semaphores(list(self.sems.allocated().values()))
        nc.all_engine_barrier()

    tile.TileContext._add_instruction = _add_split
    tile.TileContext._drain_and_barrier = _drain_and_barrier_split
    tile.TileContext._wait_split_patched = True


# ---------------------------------------------------------------- builder
def _build_nc(S, F, mae_rows):
    import concourse.bass as bass
    import concourse.mybir as mybir
    from concourse import tile

    _apply_walrus_patches()

    f32 = mybir.dt.float32
    bf16 = mybir.dt.bfloat16
    AL = mybir.AluOpType
    AF = mybir.ActivationFunctionType

    W, BW, SL = _W, _BW, _SLOPE
    NB = S // W                          # 128 column chunks
    NSTEP = S + SL * (NB - 1)            # 2429 wavefront steps
    NBLK = NSTEP + SL                    # absolute 17-col blocks
    DSKC = BW * NBLK                     # dsk free cols (41344)
    HEADC = BW * (SL * NB)               # head poison cols [0, 6528)
    TAILC = BW * (SL + S)                # tail poison from col 34867
    NCH = S // 128                       # 16 derivation chunks

    nc = bass.Bass("TRN2", target_bir_lowering=False, debug=False, num_devices=8)

    mp_d = nc.dram_tensor("mp", [mae_rows, F], bf16, kind="ExternalInput").ap()
    mt_d = nc.dram_tensor("mt", [mae_rows, F], bf16, kind="ExternalInput").ap()
    dtw_d = nc.dram_tensor("dtw", [1, 1], f32, kind="ExternalOutput").ap()
    mae_d = nc.dram_tensor("mae", [1, 1], f32, kind="ExternalOutput").ap()

    mae_free = mae_rows * F // 128       # 4096 cols per partition (bf16)
    MCH = 512
    n_mch = mae_free // MCH              # 8 chunks

    with tile.TileContext(nc) as tc:
        with (
            tc.tile_pool(name="big", bufs=1) as bp,
            tc.tile_pool(name="stg", bufs=2) as stgp,
            tc.tile_pool(name="mch", bufs=4) as mchp,
            tc.tile_pool(name="dfp", bufs=2) as dfp,
            tc.tile_pool(name="red", bufs=2) as redp,
            tc.tile_pool(name="xcp", bufs=4) as xcp,
            tc.tile_pool(name="sqp", bufs=4) as sqp,
            tc.tile_pool(name="gps", bufs=2, space=bass.MemorySpace.PSUM) as gpsp,
            tc.tile_pool(name="sps", bufs=4, space=bass.MemorySpace.PSUM) as spsp,
        ):
            dsk = bp.tile([128, DSKC], f32, tag="dsk")
            XTN = bp.tile([128, S], bf16, tag="XTN")
            YT = bp.tile([128, S], bf16, tag="YT")
            xq = bp.tile([2, S], bf16, tag="xq")
            yq = bp.tile([2, S], bf16, tag="yq")
            mB = bp.tile([128, BW], f32, tag="mB")
            shT = bp.tile([128, 128], f32, tag="shT")
            eye = bp.tile([128, 128], bf16, tag="eye")
            brow = bp.tile([1, 128], f32, tag="brow")
            otwo = bp.tile([1, 2], f32, tag="otwo")
            ones128 = bp.tile([128, 1], f32, tag="ones128")
            macc = bp.tile([128, 1], f32, tag="macc")
            mres = bp.tile([1, 1], f32, tag="mres")

            # ---------------- prologue: consts + poison -----------------
            nc.gpsimd.memset(ones128[:], 1.0)
            nc.gpsimd.memset(macc[:], 0.0)
            nc.gpsimd.memset(mB[:, 0:1], _BIG)
            nc.gpsimd.memset(brow[:], 0.0)
            nc.gpsimd.memset(brow[0:1, 0:1], _BIG)
            nc.gpsimd.memset(otwo[:], 1.0)
            nc.gpsimd.memset(eye[:], 1.0)
            nc.gpsimd.memset(shT[:], 1.0)
            # engine writes must start at partition 0: fill both rows with
            # ones, then overwrite the norm row (xq row0 via ACT; yq row1 is
            # at partition 1, so it goes through a partition-0 staging tile
            # + DMA, which has no partition-start restriction)
            nc.gpsimd.memset(xq[:, :], 1.0)
            nc.gpsimd.memset(yq[:, :], 1.0)
            # eye[p,j] = (j-p == 0), shT[p,j] = (j-p-1 == 0)
            nc.gpsimd.affine_select(
                eye[:], eye[:], [[1, 128]], AL.is_equal, 0.0,
                base=0, channel_multiplier=-1,
            )
            nc.gpsimd.affine_select(
                shT[:], shT[:], [[1, 128]], AL.is_equal, 0.0,
                base=-1, channel_multiplier=-1,
            )
            nc.vector.memset(dsk[:, 0:HEADC], _BIG)
            nc.gpsimd.memset(dsk[:, TAILC:DSKC], _BIG)
            # DP start cell: block(-1)[chunk0, col0] = 0 enables D[0,0]=d[0,0]
            nc.gpsimd.memset(dsk[0:1, (SL - 1) * BW:(SL - 1) * BW + 1], 0.0)

            # zero col0 of all 32 chunk-groups in both stg buffers (the skew
            # DMA carries them into block col0 = the scan's "d=0" halo slot)
            stg_tiles = []
            for _ in range(2):
                s = stgp.tile([128, 32 * BW], f32, tag="stg")
                for cl in range(32):
                    nc.gpsimd.memset(s[:, cl * BW:cl * BW + 1], 0.0)
                stg_tiles.append(s)

            # ---------------- on-device DTW input derivation ------------
            # x = rows [0,2048) of this core's mp shard (= pred[0] on core 0),
            # y = rows [0,2048) of mt.  Per 128-row chunk c, one PSUM bank:
            #   PE: pg[:,0:128] = Xc^T, pg[:,128:256] = Yc^T (identity mm)
            #   ACT: XTN[:,c] = -2*Xc^T (bf16), YT[:,c] = Yc^T (bf16),
            #        sqA/sqB = Square(transposes) (f32 SBUF)
            #   PE: pg[0,256:384] = ones^T @ sqA = |x_s|^2 row (same for y)
            #   ACT: xq[0, c*128:+128] / yq[1, ...] = norm rows (bf16)
            for c in range(NCH):
                r0 = 128 * c
                Xc = xcp.tile([128, 128], bf16, tag="Xc")
                Yc = xcp.tile([128, 128], bf16, tag="Yc")
                nc.sync.dma_start(Xc[:], mp_d[r0:r0 + 128, :])
                nc.sync.dma_start(Yc[:], mt_d[r0:r0 + 128, :])
                pg = gpsp.tile([128, 512], f32, tag="pg")
                nc.tensor.matmul(pg[:, 0:128], Xc[:], eye[:], start=True, stop=True)
                nc.tensor.matmul(pg[:, 128:256], Yc[:], eye[:], start=True, stop=True)
                nc.scalar.activation(
                    XTN[:, r0:r0 + 128], pg[:, 0:128], AF.Copy, scale=-2.0
                )
                nc.scalar.activation(YT[:, r0:r0 + 128], pg[:, 128:256], AF.Copy)
                sqA = sqp.tile([128, 128], f32, tag="sqA")
                sqB = sqp.tile([128, 128], f32, tag="sqB")
                nc.scalar.activation(sqA[:], pg[:, 0:128], AF.Square)
                nc.scalar.activation(sqB[:], pg[:, 128:256], AF.Square)
                nc.tensor.matmul(
                    pg[0:1, 256:384], ones128[:], sqA[:], start=True, stop=True
                )
                nc.tensor.matmul(
                    pg[0:1, 384:512], ones128[:], sqB[:], start=True, stop=True
                )
                nc.scalar.activation(xq[0:1, r0:r0 + 128], pg[0:1, 256:384], AF.Copy)
                yst = xcp.tile([1, 128], bf16, tag="yst")
                nc.scalar.activation(yst[:], pg[0:1, 384:512], AF.Copy)
                nc.gpsimd.dma_start(yq[1:2, r0:r0 + 128], yst[:])

            # ---------------- d production ------------------------------
            # block (ib, jc): DP rows [128ib, 128ib+128) x cols [512jc, +512)
            def emit_mm(pg, ib, jc, sl):
                i0, a = 128 * ib, 512 * jc + 128 * sl
                nc.tensor.matmul(
                    pg[:, 128 * sl:128 * sl + 128],
                    XTN[:, i0:i0 + 128], YT[:, a:a + 128],
                    start=True, stop=False,
                )
                nc.tensor.matmul(
                    pg[:, 128 * sl:128 * sl + 128],
                    xq[:, i0:i0 + 128], yq[:, a:a + 128],
                    start=False, stop=True,
                )

            def emit_sqrt(pg, st, sl):
                gap = bass.AP(
                    st.tensor, BW * 8 * sl + 1,
                    [[32 * BW, 128], [BW, 8], [1, W]],
                )
                nc.scalar.activation(gap, pg[:, 128 * sl:128 * sl + 128], AF.Sqrt)

            def emit_dma(st, ib, jc, h):
                # 8 per-chunk DMAs (BIR APs cannot step partitions off-pitch);
                # issued from the Pool queue whose DMA dispatch is ~20x
                # cheaper than SP's
                i0 = 128 * ib
                for cl in range(8 * h, 8 * h + 8):
                    c = 32 * jc + cl
                    src = st[:, cl * BW:(cl + 1) * BW]
                    dst = bass.AP(
                        dsk.tensor,
                        c * DSKC + (i0 + SL * c + SL) * BW,
                        [[DSKC, 1], [BW, 128], [1, BW]],
                    )
                    nc.gpsimd.dma_start(dst, src)

            def emit_block(ib, jc):
                pg = gpsp.tile([128, 512], f32, tag="pg")
                st = stg_tiles[0]
                stg_tiles.reverse()
                for sl in range(4):
                    emit_mm(pg, ib, jc, sl)
                for sl in range(4):
                    emit_sqrt(pg, st, sl)
                for h in range(4):
                    emit_dma(st, ib, jc, h)

            # first 3 i-blocks fully in the prologue (needed from step 0)
            for ib in range(3):
                for jc in range(4):
                    emit_block(ib, jc)

            # remaining 52 blocks paced into the wavefront, deadline order.
            # each block = 10 slots (4 mm-pairs, 4 sqrts, 2 dma) 3 steps apart
            rest = sorted(
                [(ib, jc) for ib in range(3, S // 128) for jc in range(4)],
                key=lambda b: 128 * b[0] + 96 * b[1],
            )
            sched = {}
            blk_state = {}
            for n, (ib, jc) in enumerate(rest):
                base = 10 + 32 * n
                for s in range(4):
                    sched.setdefault(base + 3 * s, []).append(("mm", ib, jc, s))
                for s in range(4):
                    sched.setdefault(base + 12 + 3 * s, []).append(("sq", ib, jc, s))
                for h in range(4):
                    sched.setdefault(base + 22 + 2 * h, []).append(("dm", ib, jc, h))

            # MAE chunk k: prefetch DMA at E_k, DVE consume at E_k + 220
            mpv = mp_d.rearrange("(p x) f -> p (x f)", p=128)
            mtv = mt_d.rearrange("(p x) f -> p (x f)", p=128)
            mae_tiles = {}
            for k in range(n_mch):
                sched.setdefault(120 + 260 * k, []).append(("ml", k))
                sched.setdefault(340 + 260 * k, []).append(("mc", k))

            def emit_sched(t):
                for op in sched.pop(t, ()):
                    if op[0] == "mm":
                        _, ib, jc, sl = op
                        if sl == 0:
                            pg = gpsp.tile([128, 512], f32, tag="pg")
                            st = stg_tiles[0]
                            stg_tiles.reverse()
                            blk_state[(ib, jc)] = (pg, st)
                        pg, st = blk_state[(ib, jc)]
                        emit_mm(pg, ib, jc, sl)
                    elif op[0] == "sq":
                        _, ib, jc, sl = op
                        pg, st = blk_state[(ib, jc)]
                        emit_sqrt(pg, st, sl)
                    elif op[0] == "dm":
                        _, ib, jc, h = op
                        pg, st = blk_state[(ib, jc)]
                        emit_dma(st, ib, jc, h)
                    elif op[0] == "ml":
                        k = op[1]
                        ta = mchp.tile([128, MCH], bf16, tag="ma")
                        tb = mchp.tile([128, MCH], bf16, tag="mb")
                        nc.sync.dma_start(ta[:], mpv[:, k * MCH:(k + 1) * MCH])
                        nc.sync.dma_start(tb[:], mtv[:, k * MCH:(k + 1) * MCH])
                        mae_tiles[k] = (ta, tb)
                    else:  # "mc"
                        k = op[1]
                        ta, tb = mae_tiles.pop(k)
                        df = dfp.tile([128, MCH], f32, tag="df")
                        nc.vector.tensor_tensor(df[:], ta[:], tb[:], AL.subtract)
                        red = redp.tile([128, 1], f32, tag="red")
                        nc.vector.tensor_reduce(
                            red[:], df[:], mybir.AxisListType.X, AL.add,
                            apply_absolute_value=True,
                        )
                        nc.vector.tensor_tensor(macc[:], macc[:], red[:], AL.add)

            # ---------------- wavefront ---------------------------------
            cur_ps = None
            for t in range(NSTEP):
                if t % 2 == 0:
                    # seeds for steps t, t+1: pst[c] = BIG@c0 + D-col16 of
                    # blocks (t-3),(t-2) shifted down one partition
                    cur_ps = spsp.tile([128, 2], f32, tag="pst")
                    nc.tensor.matmul(
                        cur_ps[:, 0:2], brow[:, 0:128], otwo[:],
                        start=True, stop=False,
                    )
                    nc.tensor.matmul(
                        cur_ps[:, 0:2], shT[:, 0:128],
                        dsk[:, BW * t + W:BW * t + W + 2 * BW:BW],
                        start=False, stop=True,
                    )
                emit_sched(t)
                o = BW * (t + SL)
                prev = dsk[:, o - BW:o]
                nc.vector.tensor_tensor(
                    mB[:, 1:BW], prev[:, 0:W], prev[:, 1:BW], AL.min
                )
                for _ in range(_SCAN_REPS):
                    nc.vector.tensor_tensor_scan(
                        dsk[:, o:o + BW], mB[:, 0:BW], dsk[:, o:o + BW],
                        cur_ps[:, (t % 2):(t % 2) + 1], AL.min, AL.add,
                    )
            # leftover sched events (none expected, but don't drop any)
            for t in sorted(list(sched.keys())):
                emit_sched(t)

            # ---------------- outputs -----------------------------------
            nc.sync.dma_start(dtw_d, dsk[NB - 1:NB, DSKC - 1:DSKC])
            pm = gpsp.tile([128, 512], f32, tag="pg")
            nc.tensor.matmul(
                pm[0:1, 0:1], macc[:, 0:1], ones128[:, 0:1],
                start=True, stop=True,
            )
            nc.scalar.copy(mres[0:1, 0:1], pm[0:1, 0:1])
            nc.sync.dma_start(mae_d, mres[0:1, 0:1])

    return nc


# ---------------------------------------------------------------- runtime
_CACHE = {}


def _get_nc(S, F, mae_rows):
    key = (S, F, mae_rows)
    if key not in _CACHE:
        _CACHE[key] = _build_nc(S, F, mae_rows)
    return _CACHE[key]


class _Runner:
    """Runs nc on 8 cores.  First call goes through run_bass_kernel_spmd
    (compile + validate); later calls reuse a cached compiled executable of
    the identical shard_map computation, skipping the ~1s re-trace/re-lower
    that run_bass_kernel_spmd pays per call."""

    def __init__(self, nc, n_cores=8):
        import concourse.mybir as mybir

        self.nc = nc
        self.n_cores = n_cores
        self.compiled = None
        self.ran_spmd = False

        part = nc.partition_id_tensor.name if nc.partition_id_tensor else None
        self.partition_name = part
        in_names, out_names, out_shapes, out_dtypes = [], [], [], []
        for alloc in nc.m.functions[0].allocations:
            if not isinstance(alloc, mybir.MemoryLocationSet):
                continue
            name = alloc.memorylocations[0].name
            if alloc.kind == "ExternalInput":
                if name != part:
                    in_names.append(name)
            elif alloc.kind == "ExternalOutput":
                out_names.append(name)
                out_shapes.append(tuple(alloc.tensor_shape))
                out_dtypes.append(mybir.dt.np(alloc.dtype))
        self.in_names = in_names
        self.out_names = out_names
        self.out_shapes = out_shapes
        self.out_dtypes = out_dtypes

    def _build_compiled(self, concat_in):
        import jax
        import numpy as np
        from jax.sharding import Mesh, PartitionSpec
        from jax.experimental.shard_map import shard_map
        from concourse.bass2jax import _bass_exec_p, install_neuronx_cc_hook

        install_neuronx_cc_hook()
        nc, n_cores = self.nc, self.n_cores
        out_avals = tuple(
            jax.core.ShapedArray(s, d)
            for s, d in zip(self.out_shapes, self.out_dtypes)
        )
        all_names = list(self.in_names) + list(self.out_names)
        if self.partition_name is not None:
            all_names.append(self.partition_name)
        n_params, n_outs = len(self.in_names), len(self.out_names)
        out_names = tuple(self.out_names)
        partition_name = self.partition_name

        def _body(*args):
            operands = list(args)
            if partition_name is not None:
                from concourse.bass2jax import partition_id_tensor

                operands.append(partition_id_tensor())
            outs = _bass_exec_p.bind(
                *operands,
                out_avals=out_avals,
                in_names=tuple(all_names),
                out_names=out_names,
                lowering_input_output_aliases=(),
                sim_require_finite=True,
                sim_require_nnan=True,
                nc=nc,
            )
            return tuple(outs)

        devices = jax.devices()[:n_cores]
        mesh = Mesh(np.asarray(devices), ("core",))
        in_specs = (PartitionSpec("core"),) * (n_params + n_outs)
        out_specs = (PartitionSpec("core"),) * n_outs
        donate = tuple(range(n_params, n_params + n_outs))
        sharded = jax.jit(
            shard_map(
                _body, mesh=mesh, in_specs=in_specs, out_specs=out_specs,
                check_rep=False,
            ),
            donate_argnums=donate,
            keep_unused=True,
        )
        lowered = sharded.lower(*concat_in, *self._concat_zeros())
        self.compiled = lowered.compile()

    def _concat_zeros(self):
        return [
            np.zeros((self.n_cores * s[0], *s[1:]), d)
            for s, d in zip(self.out_shapes, self.out_dtypes)
        ]

    def _run_fast(self, concat_in):
        import jax

        if self.compiled is None:
            self._build_compiled(concat_in)
        out_arrs = self.compiled(*concat_in, *self._concat_zeros())
        jax.block_until_ready(out_arrs)
        return [
            {
                name: np.asarray(out_arrs[i]).reshape(
                    self.n_cores, *self.out_shapes[i]
                )[c]
                for i, name in enumerate(self.out_names)
            }
            for c in range(self.n_cores)
        ]

    def run(self, concat_in):
        """concat_in: arrays in self.in_names order, axis-0-concatenated
        over cores.  Returns per-core dicts of outputs."""
        if not self.ran_spmd:
            # first call: the documented compile+run path (also warms the
            # NEFF cache for the cached fast path, which reuses the same
            # backend compile).
            from concourse.bass_utils import run_bass_kernel_spmd

            shard0 = [a.shape[0] // self.n_cores for a in concat_in]
            in_maps = [
                {
                    name: concat_in[i][c * shard0[i]:(c + 1) * shard0[i]]
                    for i, name in enumerate(self.in_names)
                }
                for c in range(self.n_cores)
            ]
            res = run_bass_kernel_spmd(
                self.nc, in_maps, core_ids=list(range(self.n_cores))
            )
            self.ran_spmd = True
            return res.results
        try:
            return self._run_fast(concat_in)
        except Exception:
            from concourse.bass_utils import run_bass_kernel_spmd

            shard0 = [a.shape[0] // self.n_cores for a in concat_in]
            in_maps = [
                {
                    name: concat_in[i][c * shard0[i]:(c + 1) * shard0[i]]
                    for i, name in enumerate(self.in_names)
                }
                for c in range(self.n_cores)
            ]
            res = run_bass_kernel_spmd(
                self.nc, in_maps, core_ids=list(range(self.n_cores))
            )
            return res.results


def _get_runner(S, F, mae_rows):
    key = ("runner", S, F, mae_rows)
    if key not in _CACHE:
        _CACHE[key] = _Runner(_get_nc(S, F, mae_rows))
    return _CACHE[key]


def _prepare(pred, target):
    """Host-side input prep: bf16 casts of the two batch tensors, laid out
    so the per-core shards are axis-0 slices (core k gets samples 2k,2k+1)."""
    import ml_dtypes

    B, S, F = pred.shape
    n_cores = 8
    mae_rows = (B // n_cores) * S
    bf = ml_dtypes.bfloat16
    mp_all = np.ascontiguousarray(pred, dtype=np.float32).reshape(B * S, F).astype(bf)
    mt_all = np.ascontiguousarray(target, dtype=np.float32).reshape(B * S, F).astype(bf)
    return mp_all, mt_all, mae_rows


def kernel(pred, target):
    pred = np.asarray(pred)
    target = np.asarray(target)
    B, S, F = pred.shape
    mp_all, mt_all, mae_rows = _prepare(pred, target)
    runner = _get_runner(S, F, mae_rows)
    results = runner.run([mp_all, mt_all])
    mae_sum = sum(float(results[k]["mae"][0, 0]) for k in range(8))
    dtw = float(results[0]["dtw"][0, 0])
    loss = 0.5 * (mae_sum / (B * S * F)) + 0.5 * (dtw / (S * F))
    return np.float32(loss)


# revision 12
# speedup vs baseline: 4.5524x; 1.1843x over previous
"""DTW loss (0.5*MAE + 0.5*DTW(pred[0],target[0])/(S*F)) on 8 TRN2 cores.

v4: same slope-3 anti-diagonal wavefront DP as v3, but the host->device
contract is slimmed to the information-theoretic minimum and the runtime
path is cached:

* Only two inputs are shipped per core: the bf16 batch shards mp/mt
  ([4096,128] each, 2MB/core).  Everything the DTW needs -- the F-major
  transposes XTN=-2*x^T / YT=y^T, the squared-norm rows xq/yq, the
  shift/identity matrices, poison rows -- is derived ON DEVICE in the
  prologue (PE transposes via identity matmul, ACT Square + ones-matmul
  for the norms, diagonal DMAs for I / shift).  This cuts the axon
  host->device transfer from 42.6MB to 16.8MB (~50MB/s tunnel).

* run_bass_kernel_spmd re-traces + re-lowers a fresh jax.jit on every
  call (~1s of host overhead).  kernel() uses it once (first call:
  compile + validate), then switches to a cached compiled executable of
  the identical shard_map computation, so steady-state calls pay only
  input transfer + execution.

The [S,S] DP is split into NB=128 column chunks of W=16, chunk c in SBUF
partition c.  At wavefront step t partition c processes DP row i = t - 3*c.
The whole row recurrence D[i,j] = d[i,j] + min(D[i-1,j-1], D[i-1,j], D[i,j-1])
is ONE hardware TensorTensorScan per step:

    state = seed;  state = (m[j] min state) add d[j]

with m[j] = min(D[i-1,j-1], D[i-1,j]) (one pair-min tensor_tensor) and
seed = D[i, 16c-1] from the left-neighbour chunk via a tiny PE shift-matmul
(batched two steps per matmul, 3 steps of slack so PE stays off the critical
path).  So the serial chain is just 2 DVE instructions per step.

Storage is a 17-column-per-step in-place ring: block(t) = dsk[:, 17*(t+3)+0:17]
holds [halo, d0..d15] and the scan overwrites it with [halo, D0..D15]
(out == data1; col0 has d=0 so out[0] = seed, materializing next step's halo).
Out-of-range regions are poisoned BIG so no masking is needed.

d[i,j] = sqrt(|x_i|^2+|y_j|^2-2 x.y) is produced on the fly: bf16 GEMM with
the squared norms folded in as a rank-2 augmented matmul -> ACT sqrt (written
with a 17-stride gapped AP) -> block-fused skew DMAs, all paced into the
wavefront's idle engine time.

The MAE term is data-parallel over the batch; each core reduces its 2-sample
shard on DVE in 8 chunks interleaved into the wavefront (DMA-prefetched).
"""

import numpy as np

_BIG = 1.0e30
_S, _F, _B = 2048, 128, 16
_W = 16
_BW = 17
_SLOPE = 3
_SCAN_REPS = 1  # >1 only for timing experiments (corrupts the DP)


# ---------------------------------------------------------------- patches
def _apply_walrus_patches():
    """This walrus build rejects >1 semaphore wait per instruction; peel
    extra waits onto same-engine Drain instructions."""
    import bass_rust
    import concourse.mybir as mybir
    from concourse import tile
    from concourse.vector_clock import ScopedClock

    if getattr(tile.TileContext, "_wait_split_patched", False):
        return
    _orig_add = tile.TileContext._add_instruction

    def _mk_drain(nc, engine, waits):
        d = mybir.InstDrain(name=nc.get_next_instruction_name(), engine=engine)
        d.sync_info = bass_rust.SyncInfo(on_wait=list(waits), on_update=[])
        return d

    def _add_split(self, inst):
        si = inst.sync_info
        if (
            si is not None
            and si.on_wait
            and len(si.on_wait) > 1
            and inst.engine is not None
            and inst.engine != mybir.EngineType.Unassigned
        ):
            waits = list(si.on_wait)
            si.on_wait = waits[:1]
            for w in waits[1:]:
                _orig_add(self, _mk_drain(self.nc, inst.engine, [w]))
        _orig_add(self, inst)

    def _drain_and_barrier_split(self, tick_clock, wait_clock):
        nc = self.nc
        drain_inst = nc.sync.drain()
        wait_clock.add_sem_waits(
            drain_inst.ins, ScopedClock({None: tick_clock.global_clock})
        )
        si = drain_inst.ins.sync_info
        waits = list(si.on_wait) if si and si.on_wait else []
        if len(waits) > 1:
            si.on_wait = waits[:1]
            for w in waits[1:]:
                d2 = nc.sync.drain()
                d2.ins.sync_info = bass_rust.SyncInfo(on_wait=[w], on_update=[])
        nc.all_engine_barrier()
        assert self.sems is not None
        popped = nc._tile_sem_poison_stack.pop()
        assert popped is self._sem_poison
        nc.clear_and_free_semaphores(list(self.sems.allocated().values()))
        nc.all_engine_barrier()

    tile.TileContext._add_instruction = _add_split
    tile.TileContext._drain_and_barrier = _drain_and_barrier_split
    tile.TileContext._wait_split_patched = True


# ---------------------------------------------------------------- builder
def _build_nc(S, F, mae_rows):
    import concourse.bass as bass
    import concourse.mybir as mybir
    from concourse import tile

    _apply_walrus_patches()

    f32 = mybir.dt.float32
    bf16 = mybir.dt.bfloat16
    f8 = mybir.dt.float8e4
    AL = mybir.AluOpType
    AF = mybir.ActivationFunctionType

    W, BW, SL = _W, _BW, _SLOPE
    NB = S // W                          # 128 column chunks
    NSTEP = S + SL * (NB - 1)            # 2429 wavefront steps
    NBLK = NSTEP + SL                    # absolute 17-col blocks
    DSKC = BW * NBLK                     # dsk free cols (41344)
    HEADC = BW * (SL * NB)               # head poison cols [0, 6528)
    TAILC = BW * (SL + S)                # tail poison from col 34867
    NCH = S // 128                       # 16 derivation chunks

    nc = bass.Bass("TRN2", target_bir_lowering=False, debug=False, num_devices=8)

    # one fp8 input: rows [0,mae_rows) = pred shard, [mae_rows,2m) = target
    md_d = nc.dram_tensor("md", [2 * mae_rows, F], f8, kind="ExternalInput").ap()
    mp_d = md_d[0:mae_rows, :]
    mt_d = md_d[mae_rows:2 * mae_rows, :]
    dtw_d = nc.dram_tensor("dtw", [1, 1], f32, kind="ExternalOutput").ap()
    mae_d = nc.dram_tensor("mae", [1, 1], f32, kind="ExternalOutput").ap()

    mae_free = mae_rows * F // 128       # 4096 cols per partition
    MCH = 512
    n_mch = mae_free // MCH              # 8 chunks

    with tile.TileContext(nc) as tc:
        with (
            tc.tile_pool(name="big", bufs=1) as bp,
            tc.tile_pool(name="stg", bufs=2) as stgp,
            tc.tile_pool(name="mch", bufs=4) as mchp,
            tc.tile_pool(name="dfp", bufs=2) as dfp,
            tc.tile_pool(name="red", bufs=2) as redp,
            tc.tile_pool(name="xcp", bufs=4) as xcp,
            tc.tile_pool(name="sqp", bufs=4) as sqp,
            tc.tile_pool(name="gps", bufs=2, space=bass.MemorySpace.PSUM) as gpsp,
            tc.tile_pool(name="sps", bufs=4, space=bass.MemorySpace.PSUM) as spsp,
        ):
            dsk = bp.tile([128, DSKC], f32, tag="dsk")
            XTN = bp.tile([128, S], bf16, tag="XTN")
            YT = bp.tile([128, S], bf16, tag="YT")
            xq = bp.tile([2, S], bf16, tag="xq")
            yq = bp.tile([2, S], bf16, tag="yq")
            mB = bp.tile([128, BW], f32, tag="mB")
            shT = bp.tile([128, 128], f32, tag="shT")
            eye = bp.tile([128, 128], f8, tag="eye")
            brow = bp.tile([1, 128], f32, tag="brow")
            otwo = bp.tile([1, 2], f32, tag="otwo")
            ones128 = bp.tile([128, 1], f32, tag="ones128")
            macc = bp.tile([128, 1], f32, tag="macc")
            mres = bp.tile([1, 1], f32, tag="mres")

            # ---------------- prologue: consts + poison -----------------
            nc.gpsimd.memset(ones128[:], 1.0)
            nc.gpsimd.memset(macc[:], 0.0)
            nc.gpsimd.memset(mB[:, 0:1], _BIG)
            nc.gpsimd.memset(brow[:], 0.0)
            nc.gpsimd.memset(brow[0:1, 0:1], _BIG)
            nc.gpsimd.memset(otwo[:], 1.0)
            nc.gpsimd.memset(eye[:], 1.0)
            nc.gpsimd.memset(shT[:], 1.0)
            # engine writes must start at partition 0: fill both rows with
            # ones, then overwrite the norm row (xq row0 via ACT; yq row1 is
            # at partition 1, so it goes through a partition-0 staging tile
            # + DMA, which has no partition-start restriction)
            nc.gpsimd.memset(xq[:, :], 1.0)
            nc.gpsimd.memset(yq[:, :], 1.0)
            # eye[p,j] = (j-p == 0), shT[p,j] = (j-p-1 == 0)
            nc.gpsimd.affine_select(
                eye[:], eye[:], [[1, 128]], AL.is_equal, 0.0,
                base=0, channel_multiplier=-1,
            )
            nc.gpsimd.affine_select(
                shT[:], shT[:], [[1, 128]], AL.is_equal, 0.0,
                base=-1, channel_multiplier=-1,
            )
            nc.vector.memset(dsk[:, 0:HEADC], _BIG)
            nc.gpsimd.memset(dsk[:, TAILC:DSKC], _BIG)
            # DP start cell: block(-1)[chunk0, col0] = 0 enables D[0,0]=d[0,0]
            nc.gpsimd.memset(dsk[0:1, (SL - 1) * BW:(SL - 1) * BW + 1], 0.0)

            # zero col0 of all 32 chunk-groups in both stg buffers (the skew
            # DMA carries them into block col0 = the scan's "d=0" halo slot)
            stg_tiles = []
            for _ in range(2):
                s = stgp.tile([128, 32 * BW], f32, tag="stg")
                for cl in range(32):
                    nc.gpsimd.memset(s[:, cl * BW:cl * BW + 1], 0.0)
                stg_tiles.append(s)

            # ---------------- on-device DTW input derivation ------------
            # x = rows [0,2048) of this core's mp shard (= pred[0] on core 0),
            # y = rows [0,2048) of mt.  Per 128-row chunk c, one PSUM bank:
            #   PE: pg[:,0:128] = Xc^T, pg[:,128:256] = Yc^T (identity mm)
            #   ACT: XTN[:,c] = -2*Xc^T (bf16), YT[:,c] = Yc^T (bf16),
            #        sqA/sqB = Square(transposes) (f32 SBUF)
            #   PE: pg[0,256:384] = ones^T @ sqA = |x_s|^2 row (same for y)
            #   ACT: xq[0, c*128:+128] / yq[1, ...] = norm rows (bf16)
            for c in range(NCH):
                r0 = 128 * c
                Xc = xcp.tile([128, 128], f8, tag="Xc")
                Yc = xcp.tile([128, 128], f8, tag="Yc")
                nc.sync.dma_start(Xc[:], mp_d[r0:r0 + 128, :])
                nc.sync.dma_start(Yc[:], mt_d[r0:r0 + 128, :])
                pg = gpsp.tile([128, 512], f32, tag="pg")
                nc.tensor.matmul(pg[:, 0:128], Xc[:], eye[:], start=True, stop=True)
                nc.tensor.matmul(pg[:, 128:256], Yc[:], eye[:], start=True, stop=True)
                nc.scalar.activation(
                    XTN[:, r0:r0 + 128], pg[:, 0:128], AF.Copy, scale=-2.0
                )
                nc.scalar.activation(YT[:, r0:r0 + 128], pg[:, 128:256], AF.Copy)
                sqA = sqp.tile([128, 128], f32, tag="sqA")
                sqB = sqp.tile([128, 128], f32, tag="sqB")
                nc.scalar.activation(sqA[:], pg[:, 0:128], AF.Square)
                nc.scalar.activation(sqB[:], pg[:, 128:256], AF.Square)
                nc.tensor.matmul(
                    pg[0:1, 256:384], ones128[:], sqA[:], start=True, stop=True
                )
                nc.tensor.matmul(
                    pg[0:1, 384:512], ones128[:], sqB[:], start=True, stop=True
                )
                nc.scalar.activation(xq[0:1, r0:r0 + 128], pg[0:1, 256:384], AF.Copy)
                yst = xcp.tile([1, 128], bf16, tag="yst")
                nc.scalar.activation(yst[:], pg[0:1, 384:512], AF.Copy)
                nc.gpsimd.dma_start(yq[1:2, r0:r0 + 128], yst[:])

            # ---------------- d production ------------------------------
            # block (ib, jc): DP rows [128ib, 128ib+128) x cols [512jc, +512)
            def emit_mm(pg, ib, jc, sl):
                i0, a = 128 * ib, 512 * jc + 128 * sl
                nc.tensor.matmul(
                    pg[:, 128 * sl:128 * sl + 128],
                    XTN[:, i0:i0 + 128], YT[:, a:a + 128],
                    start=True, stop=False,
                )
                nc.tensor.matmul(
                    pg[:, 128 * sl:128 * sl + 128],
                    xq[:, i0:i0 + 128], yq[:, a:a + 128],
                    start=False, stop=True,
                )

            def emit_sqrt(pg, st, sl):
                gap = bass.AP(
                    st.tensor, BW * 8 * sl + 1,
                    [[32 * BW, 128], [BW, 8], [1, W]],
                )
                nc.scalar.activation(gap, pg[:, 128 * sl:128 * sl + 128], AF.Sqrt)

            def emit_dma(st, ib, jc, h):
                # 8 per-chunk DMAs (BIR APs cannot step partitions off-pitch);
                # issued from the Pool queue whose DMA dispatch is ~20x
                # cheaper than SP's
                i0 = 128 * ib
                for cl in range(8 * h, 8 * h + 8):
                    c = 32 * jc + cl
                    src = st[:, cl * BW:(cl + 1) * BW]
                    dst = bass.AP(
                        dsk.tensor,
                        c * DSKC + (i0 + SL * c + SL) * BW,
                        [[DSKC, 1], [BW, 128], [1, BW]],
                    )
                    nc.gpsimd.dma_start(dst, src)

            def emit_block(ib, jc):
                pg = gpsp.tile([128, 512], f32, tag="pg")
                st = stg_tiles[0]
                stg_tiles.reverse()
                for sl in range(4):
                    emit_mm(pg, ib, jc, sl)
                for sl in range(4):
                    emit_sqrt(pg, st, sl)
                for h in range(4):
                    emit_dma(st, ib, jc, h)

            # first 3 i-blocks fully in the prologue (needed from step 0)
            for ib in range(3):
                for jc in range(4):
                    emit_block(ib, jc)

            # remaining 52 blocks paced into the wavefront, deadline order.
            # each block = 10 slots (4 mm-pairs, 4 sqrts, 2 dma) 3 steps apart
            rest = sorted(
                [(ib, jc) for ib in range(3, S // 128) for jc in range(4)],
                key=lambda b: 128 * b[0] + 96 * b[1],
            )
            sched = {}
            blk_state = {}
            for n, (ib, jc) in enumerate(rest):
                base = 10 + 32 * n
                for s in range(4):
                    sched.setdefault(base + 3 * s, []).append(("mm", ib, jc, s))
                for s in range(4):
                    sched.setdefault(base + 12 + 3 * s, []).append(("sq", ib, jc, s))
                for h in range(4):
                    sched.setdefault(base + 22 + 2 * h, []).append(("dm", ib, jc, h))

            # MAE chunk k: prefetch DMA at E_k, DVE consume at E_k + 220
            mpv = mp_d.rearrange("(p x) f -> p (x f)", p=128)
            mtv = mt_d.rearrange("(p x) f -> p (x f)", p=128)
            mae_tiles = {}
            for k in range(n_mch):
                sched.setdefault(120 + 260 * k, []).append(("ml", k))
                sched.setdefault(340 + 260 * k, []).append(("mc", k))

            def emit_sched(t):
                for op in sched.pop(t, ()):
                    if op[0] == "mm":
                        _, ib, jc, sl = op
                        if sl == 0:
                            pg = gpsp.tile([128, 512], f32, tag="pg")
                            st = stg_tiles[0]
                            stg_tiles.reverse()
                            blk_state[(ib, jc)] = (pg, st)
                        pg, st = blk_state[(ib, jc)]
                        emit_mm(pg, ib, jc, sl)
                    elif op[0] == "sq":
                        _, ib, jc, sl = op
                        pg, st = blk_state[(ib, jc)]
                        emit_sqrt(pg, st, sl)
                    elif op[0] == "dm":
                        _, ib, jc, h = op
                        pg, st = blk_state[(ib, jc)]
                        emit_dma(st, ib, jc, h)
                    elif op[0] == "ml":
                        k = op[1]
                        ta = mchp.tile([128, MCH], f8, tag="ma")
                        tb = mchp.tile([128, MCH], f8, tag="mb")
                        nc.sync.dma_start(ta[:], mpv[:, k * MCH:(k + 1) * MCH])
                        nc.sync.dma_start(tb[:], mtv[:, k * MCH:(k + 1) * MCH])
                        mae_tiles[k] = (ta, tb)
                    else:  # "mc"
                        k = op[1]
                        ta, tb = mae_tiles.pop(k)
                        df = dfp.tile([128, MCH], f32, tag="df")
                        nc.vector.tensor_tensor(df[:], ta[:], tb[:], AL.subtract)
                        red = redp.tile([128, 1], f32, tag="red")
                        nc.vector.tensor_reduce(
                            red[:], df[:], mybir.AxisListType.X, AL.add,
                            apply_absolute_value=True,
                        )
                        nc.vector.tensor_tensor(macc[:], macc[:], red[:], AL.add)

            # ---------------- wavefront ---------------------------------
            cur_ps = None
            for t in range(NSTEP):
                if t % 2 == 0:
                    # seeds for steps t, t+1: pst[c] = BIG@c0 + D-col16 of
                    # blocks (t-3),(t-2) shifted down one partition
                    cur_ps = spsp.tile([128, 2], f32, tag="pst")
                    nc.tensor.matmul(
                        cur_ps[:, 0:2], brow[:, 0:128], otwo[:],
                        start=True, stop=False,
                    )
                    nc.tensor.matmul(
                        cur_ps[:, 0:2], shT[:, 0:128],
                        dsk[:, BW * t + W:BW * t + W + 2 * BW:BW],
                        start=False, stop=True,
                    )
                emit_sched(t)
                o = BW * (t + SL)
                prev = dsk[:, o - BW:o]
                nc.vector.tensor_tensor(
                    mB[:, 1:BW], prev[:, 0:W], prev[:, 1:BW], AL.min
                )
                for _ in range(_SCAN_REPS):
                    nc.vector.tensor_tensor_scan(
                        dsk[:, o:o + BW], mB[:, 0:BW], dsk[:, o:o + BW],
                        cur_ps[:, (t % 2):(t % 2) + 1], AL.min, AL.add,
                    )
            # leftover sched events (none expected, but don't drop any)
            for t in sorted(list(sched.keys())):
                emit_sched(t)

            # ---------------- outputs -----------------------------------
            nc.sync.dma_start(dtw_d, dsk[NB - 1:NB, DSKC - 1:DSKC])
            pm = gpsp.tile([128, 512], f32, tag="pg")
            nc.tensor.matmul(
                pm[0:1, 0:1], macc[:, 0:1], ones128[:, 0:1],
                start=True, stop=True,
            )
            nc.scalar.copy(mres[0:1, 0:1], pm[0:1, 0:1])
            nc.sync.dma_start(mae_d, mres[0:1, 0:1])

    return nc


# ---------------------------------------------------------------- runtime
_CACHE = {}


def _get_nc(S, F, mae_rows):
    key = (S, F, mae_rows)
    if key not in _CACHE:
        _CACHE[key] = _build_nc(S, F, mae_rows)
    return _CACHE[key]


class _Runner:
    """Runs nc on 8 cores.  First call goes through run_bass_kernel_spmd
    (compile + validate); later calls reuse a cached compiled executable of
    the identical shard_map computation, skipping the ~1s re-trace/re-lower
    that run_bass_kernel_spmd pays per call."""

    def __init__(self, nc, n_cores=8):
        import concourse.mybir as mybir

        self.nc = nc
        self.n_cores = n_cores
        self.compiled = None
        self.ran_spmd = False

        part = nc.partition_id_tensor.name if nc.partition_id_tensor else None
        self.partition_name = part
        in_names, out_names, out_shapes, out_dtypes = [], [], [], []
        for alloc in nc.m.functions[0].allocations:
            if not isinstance(alloc, mybir.MemoryLocationSet):
                continue
            name = alloc.memorylocations[0].name
            if alloc.kind == "ExternalInput":
                if name != part:
                    in_names.append(name)
            elif alloc.kind == "ExternalOutput":
                out_names.append(name)
                out_shapes.append(tuple(alloc.tensor_shape))
                out_dtypes.append(mybir.dt.np(alloc.dtype))
        self.in_names = in_names
        self.out_names = out_names
        self.out_shapes = out_shapes
        self.out_dtypes = out_dtypes

    def _build_compiled(self, concat_in):
        import jax
        import numpy as np
        from jax.sharding import Mesh, PartitionSpec
        from jax.experimental.shard_map import shard_map
        from concourse.bass2jax import _bass_exec_p, install_neuronx_cc_hook

        install_neuronx_cc_hook()
        nc, n_cores = self.nc, self.n_cores
        out_avals = tuple(
            jax.core.ShapedArray(s, d)
            for s, d in zip(self.out_shapes, self.out_dtypes)
        )
        all_names = list(self.in_names) + list(self.out_names)
        if self.partition_name is not None:
            all_names.append(self.partition_name)
        n_params, n_outs = len(self.in_names), len(self.out_names)
        out_names = tuple(self.out_names)
        partition_name = self.partition_name

        def _body(*args):
            operands = list(args)
            if partition_name is not None:
                from concourse.bass2jax import partition_id_tensor

                operands.append(partition_id_tensor())
            outs = _bass_exec_p.bind(
                *operands,
                out_avals=out_avals,
                in_names=tuple(all_names),
                out_names=out_names,
                lowering_input_output_aliases=(),
                sim_require_finite=True,
                sim_require_nnan=True,
                nc=nc,
            )
            return tuple(outs)

        devices = jax.devices()[:n_cores]
        mesh = Mesh(np.asarray(devices), ("core",))
        in_specs = (PartitionSpec("core"),) * (n_params + n_outs)
        out_specs = (PartitionSpec("core"),) * n_outs
        donate = tuple(range(n_params, n_params + n_outs))
        sharded = jax.jit(
            shard_map(
                _body, mesh=mesh, in_specs=in_specs, out_specs=out_specs,
                check_rep=False,
            ),
            donate_argnums=donate,
            keep_unused=True,
        )
        lowered = sharded.lower(*concat_in, *self._concat_zeros())
        self.compiled = lowered.compile()

    def _concat_zeros(self):
        return [
            np.zeros((self.n_cores * s[0], *s[1:]), d)
            for s, d in zip(self.out_shapes, self.out_dtypes)
        ]

    def _run_fast(self, concat_in):
        import jax

        if self.compiled is None:
            self._build_compiled(concat_in)
        out_arrs = self.compiled(*concat_in, *self._concat_zeros())
        jax.block_until_ready(out_arrs)
        return [
            {
                name: np.asarray(out_arrs[i]).reshape(
                    self.n_cores, *self.out_shapes[i]
                )[c]
                for i, name in enumerate(self.out_names)
            }
            for c in range(self.n_cores)
        ]

    def run(self, concat_in):
        """concat_in: arrays in self.in_names order, axis-0-concatenated
        over cores.  Returns per-core dicts of outputs."""
        if not self.ran_spmd:
            # first call: the documented compile+run path (also warms the
            # NEFF cache for the cached fast path, which reuses the same
            # backend compile).
            from concourse.bass_utils import run_bass_kernel_spmd

            shard0 = [a.shape[0] // self.n_cores for a in concat_in]
            in_maps = [
                {
                    name: concat_in[i][c * shard0[i]:(c + 1) * shard0[i]]
                    for i, name in enumerate(self.in_names)
                }
                for c in range(self.n_cores)
            ]
            res = run_bass_kernel_spmd(
                self.nc, in_maps, core_ids=list(range(self.n_cores))
            )
            self.ran_spmd = True
            return res.results
        try:
            return self._run_fast(concat_in)
        except Exception:
            from concourse.bass_utils import run_bass_kernel_spmd

            shard0 = [a.shape[0] // self.n_cores for a in concat_in]
            in_maps = [
                {
                    name: concat_in[i][c * shard0[i]:(c + 1) * shard0[i]]
                    for i, name in enumerate(self.in_names)
                }
                for c in range(self.n_cores)
            ]
            res = run_bass_kernel_spmd(
                self.nc, in_maps, core_ids=list(range(self.n_cores))
            )
            return res.results


def _get_runner(S, F, mae_rows):
    key = ("runner", S, F, mae_rows)
    if key not in _CACHE:
        _CACHE[key] = _Runner(_get_nc(S, F, mae_rows))
    return _CACHE[key]


_FP8_LUT = None


def _to_fp8_bits(x32):
    """f32 -> fp8e4m3 via RTNE-to-bf16 then a 64K LUT (2x faster than
    ml_dtypes' direct cast; the double rounding moves only exact-halfway
    cases by <=1 fp8 ulp)."""
    import ml_dtypes

    global _FP8_LUT
    if _FP8_LUT is None:
        _FP8_LUT = (
            np.arange(65536, dtype=np.uint16)
            .view(ml_dtypes.bfloat16)
            .astype(ml_dtypes.float8_e4m3)
            .view(np.uint8)
        )
    return _FP8_LUT[x32.astype(ml_dtypes.bfloat16).view(np.uint16)]


def _prepare(pred, target):
    """Host-side input prep: one fp8 tensor whose per-core axis-0 slice is
    [pred shard (4096 rows); target shard (4096 rows)]."""
    import ml_dtypes

    B, S, F = pred.shape
    n_cores = 8
    shard = B // n_cores
    mae_rows = shard * S
    pred = np.ascontiguousarray(pred, dtype=np.float32)
    target = np.ascontiguousarray(target, dtype=np.float32)
    md = np.empty((n_cores, 2 * mae_rows, F), np.uint8)
    md[:, :mae_rows] = _to_fp8_bits(pred.reshape(n_cores, mae_rows, F))
    md[:, mae_rows:] = _to_fp8_bits(target.reshape(n_cores, mae_rows, F))
    md_all = md.reshape(n_cores * 2 * mae_rows, F).view(ml_dtypes.float8_e4m3)
    return md_all, mae_rows


def kernel(pred, target):
    pred = np.asarray(pred)
    target = np.asarray(target)
    B, S, F = pred.shape
    md_all, mae_rows = _prepare(pred, target)
    runner = _get_runner(S, F, mae_rows)
    results = runner.run([md_all])
    mae_sum = sum(float(results[k]["mae"][0, 0]) for k in range(8))
    dtw = float(results[0]["dtw"][0, 0])
    loss = 0.5 * (mae_sum / (B * S * F)) + 0.5 * (dtw / (S * F))
    return np.float32(loss)


# revision 13
# speedup vs baseline: 5.3141x; 1.1673x over previous
"""DTW loss (0.5*MAE + 0.5*DTW(pred[0],target[0])/(S*F)) on 8 TRN2 cores.

v4: same slope-3 anti-diagonal wavefront DP as v3, but the host->device
contract is slimmed to the information-theoretic minimum and the runtime
path is cached:

* Only two inputs are shipped per core: the bf16 batch shards mp/mt
  ([4096,128] each, 2MB/core).  Everything the DTW needs -- the F-major
  transposes XTN=-2*x^T / YT=y^T, the squared-norm rows xq/yq, the
  shift/identity matrices, poison rows -- is derived ON DEVICE in the
  prologue (PE transposes via identity matmul, ACT Square + ones-matmul
  for the norms, diagonal DMAs for I / shift).  This cuts the axon
  host->device transfer from 42.6MB to 16.8MB (~50MB/s tunnel).

* run_bass_kernel_spmd re-traces + re-lowers a fresh jax.jit on every
  call (~1s of host overhead).  kernel() uses it once (first call:
  compile + validate), then switches to a cached compiled executable of
  the identical shard_map computation, so steady-state calls pay only
  input transfer + execution.

The [S,S] DP is split into NB=128 column chunks of W=16, chunk c in SBUF
partition c.  At wavefront step t partition c processes DP row i = t - 3*c.
The whole row recurrence D[i,j] = d[i,j] + min(D[i-1,j-1], D[i-1,j], D[i,j-1])
is ONE hardware TensorTensorScan per step:

    state = seed;  state = (m[j] min state) add d[j]

with m[j] = min(D[i-1,j-1], D[i-1,j]) (one pair-min tensor_tensor) and
seed = D[i, 16c-1] from the left-neighbour chunk via a tiny PE shift-matmul
(batched two steps per matmul, 3 steps of slack so PE stays off the critical
path).  So the serial chain is just 2 DVE instructions per step.

Storage is a 17-column-per-step in-place ring: block(t) = dsk[:, 17*(t+3)+0:17]
holds [halo, d0..d15] and the scan overwrites it with [halo, D0..D15]
(out == data1; col0 has d=0 so out[0] = seed, materializing next step's halo).
Out-of-range regions are poisoned BIG so no masking is needed.

d[i,j] = sqrt(|x_i|^2+|y_j|^2-2 x.y) is produced on the fly: bf16 GEMM with
the squared norms folded in as a rank-2 augmented matmul -> ACT sqrt (written
with a 17-stride gapped AP) -> block-fused skew DMAs, all paced into the
wavefront's idle engine time.

The MAE term is data-parallel over the batch; each core reduces its 2-sample
shard on DVE in 8 chunks interleaved into the wavefront (DMA-prefetched).
"""

import numpy as np

_BIG = 1.0e30
_S, _F, _B = 2048, 128, 16
_W = 16
_BW = 17
_SLOPE = 3
_SCAN_REPS = 1  # >1 only for timing experiments (corrupts the DP)


# ---------------------------------------------------------------- patches
def _apply_walrus_patches():
    """This walrus build rejects >1 semaphore wait per instruction; peel
    extra waits onto same-engine Drain instructions."""
    import bass_rust
    import concourse.mybir as mybir
    from concourse import tile
    from concourse.vector_clock import ScopedClock

    if getattr(tile.TileContext, "_wait_split_patched", False):
        return
    _orig_add = tile.TileContext._add_instruction

    def _mk_drain(nc, engine, waits):
        d = mybir.InstDrain(name=nc.get_next_instruction_name(), engine=engine)
        d.sync_info = bass_rust.SyncInfo(on_wait=list(waits), on_update=[])
        return d

    def _add_split(self, inst):
        si = inst.sync_info
        if (
            si is not None
            and si.on_wait
            and len(si.on_wait) > 1
            and inst.engine is not None
            and inst.engine != mybir.EngineType.Unassigned
        ):
            waits = list(si.on_wait)
            si.on_wait = waits[:1]
            for w in waits[1:]:
                _orig_add(self, _mk_drain(self.nc, inst.engine, [w]))
        _orig_add(self, inst)

    def _drain_and_barrier_split(self, tick_clock, wait_clock):
        nc = self.nc
        drain_inst = nc.sync.drain()
        wait_clock.add_sem_waits(
            drain_inst.ins, ScopedClock({None: tick_clock.global_clock})
        )
        si = drain_inst.ins.sync_info
        waits = list(si.on_wait) if si and si.on_wait else []
        if len(waits) > 1:
            si.on_wait = waits[:1]
            for w in waits[1:]:
                d2 = nc.sync.drain()
                d2.ins.sync_info = bass_rust.SyncInfo(on_wait=[w], on_update=[])
        nc.all_engine_barrier()
        assert self.sems is not None
        popped = nc._tile_sem_poison_stack.pop()
        assert popped is self._sem_poison
        nc.clear_and_free_semaphores(list(self.sems.allocated().values()))
        nc.all_engine_barrier()

    tile.TileContext._add_instruction = _add_split
    tile.TileContext._drain_and_barrier = _drain_and_barrier_split
    tile.TileContext._wait_split_patched = True


# ---------------------------------------------------------------- builder
def _build_nc(S, F, mae_rows):
    import concourse.bass as bass
    import concourse.mybir as mybir
    from concourse import tile

    _apply_walrus_patches()

    f32 = mybir.dt.float32
    bf16 = mybir.dt.bfloat16
    f8 = mybir.dt.float8e4
    AL = mybir.AluOpType
    AF = mybir.ActivationFunctionType

    W, BW, SL = _W, _BW, _SLOPE
    NB = S // W                          # 128 column chunks
    NSTEP = S + SL * (NB - 1)            # 2429 wavefront steps
    NBLK = NSTEP + SL                    # absolute 17-col blocks
    DSKC = BW * NBLK                     # dsk free cols (41344)
    HEADC = BW * (SL * NB)               # head poison cols [0, 6528)
    TAILC = BW * (SL + S)                # tail poison from col 34867
    NCH = S // 128                       # 16 derivation chunks

    nc = bass.Bass("TRN2", target_bir_lowering=False, debug=False, num_devices=8)

    # one fp8 input: rows [0,mae_rows) = pred shard, [mae_rows,2m) = target
    md_d = nc.dram_tensor("md", [2 * mae_rows, F], f8, kind="ExternalInput").ap()
    mp_d = md_d[0:mae_rows, :]
    mt_d = md_d[mae_rows:2 * mae_rows, :]
    dtw_d = nc.dram_tensor("dtw", [1, 1], f32, kind="ExternalOutput").ap()
    mae_d = nc.dram_tensor("mae", [1, 1], f32, kind="ExternalOutput").ap()

    mae_free = mae_rows * F // 128       # 4096 cols per partition
    MCH = 512
    n_mch = mae_free // MCH              # 8 chunks

    with tile.TileContext(nc) as tc:
        with (
            tc.tile_pool(name="big", bufs=1) as bp,
            tc.tile_pool(name="stg", bufs=2) as stgp,
            tc.tile_pool(name="mch", bufs=4) as mchp,
            tc.tile_pool(name="dfp", bufs=2) as dfp,
            tc.tile_pool(name="red", bufs=2) as redp,
            tc.tile_pool(name="xcp", bufs=4) as xcp,
            tc.tile_pool(name="sqp", bufs=4) as sqp,
            tc.tile_pool(name="gps", bufs=2, space=bass.MemorySpace.PSUM) as gpsp,
            tc.tile_pool(name="sps", bufs=4, space=bass.MemorySpace.PSUM) as spsp,
        ):
            dsk = bp.tile([128, DSKC], f32, tag="dsk")
            XTN = bp.tile([128, S], bf16, tag="XTN")
            YT = bp.tile([128, S], bf16, tag="YT")
            xq = bp.tile([2, S], bf16, tag="xq")
            yq = bp.tile([2, S], bf16, tag="yq")
            mB = bp.tile([128, BW], f32, tag="mB")
            shT = bp.tile([128, 128], f32, tag="shT")
            eye = bp.tile([128, 128], f8, tag="eye")
            brow = bp.tile([1, 128], f32, tag="brow")
            otwo = bp.tile([1, 2], f32, tag="otwo")
            ones128 = bp.tile([128, 1], f32, tag="ones128")
            macc = bp.tile([128, 1], f32, tag="macc")
            mres = bp.tile([1, 1], f32, tag="mres")

            # ---------------- prologue: consts + poison -----------------
            nc.gpsimd.memset(ones128[:], 1.0)
            nc.gpsimd.memset(macc[:], 0.0)
            nc.gpsimd.memset(mB[:, 0:1], _BIG)
            nc.gpsimd.memset(brow[:], 0.0)
            nc.gpsimd.memset(brow[0:1, 0:1], _BIG)
            nc.gpsimd.memset(otwo[:], 1.0)
            nc.gpsimd.memset(eye[:], 1.0)
            nc.gpsimd.memset(shT[:], 1.0)
            # engine writes must start at partition 0: fill both rows with
            # ones, then overwrite the norm row (xq row0 via ACT; yq row1 is
            # at partition 1, so it goes through a partition-0 staging tile
            # + DMA, which has no partition-start restriction)
            nc.gpsimd.memset(xq[:, :], 1.0)
            nc.gpsimd.memset(yq[:, :], 1.0)
            # eye[p,j] = (j-p == 0), shT[p,j] = (j-p-1 == 0)
            nc.gpsimd.affine_select(
                eye[:], eye[:], [[1, 128]], AL.is_equal, 0.0,
                base=0, channel_multiplier=-1,
            )
            nc.gpsimd.affine_select(
                shT[:], shT[:], [[1, 128]], AL.is_equal, 0.0,
                base=-1, channel_multiplier=-1,
            )
            nc.vector.memset(dsk[:, 0:HEADC], _BIG)
            nc.gpsimd.memset(dsk[:, TAILC:DSKC], _BIG)
            # DP start cell: block(-1)[chunk0, col0] = 0 enables D[0,0]=d[0,0]
            nc.gpsimd.memset(dsk[0:1, (SL - 1) * BW:(SL - 1) * BW + 1], 0.0)

            # zero col0 of all 32 chunk-groups in both stg buffers (the skew
            # DMA carries them into block col0 = the scan's "d=0" halo slot)
            stg_tiles = []
            for _ in range(2):
                s = stgp.tile([128, 32 * BW], f32, tag="stg")
                for cl in range(32):
                    nc.gpsimd.memset(s[:, cl * BW:cl * BW + 1], 0.0)
                stg_tiles.append(s)

            # ---------------- on-device DTW input derivation ------------
            # x = rows [0,2048) of this core's mp shard (= pred[0] on core 0),
            # y = rows [0,2048) of mt.  Per 128-row chunk c, one PSUM bank:
            #   PE: pg[:,0:128] = Xc^T, pg[:,128:256] = Yc^T (identity mm)
            #   ACT: XTN[:,c] = -2*Xc^T (bf16), YT[:,c] = Yc^T (bf16),
            #        sqA/sqB = Square(transposes) (f32 SBUF)
            #   PE: pg[0,256:384] = ones^T @ sqA = |x_s|^2 row (same for y)
            #   ACT: xq[0, c*128:+128] / yq[1, ...] = norm rows (bf16)
            for c in range(NCH):
                r0 = 128 * c
                Xc = xcp.tile([128, 128], f8, tag="Xc")
                Yc = xcp.tile([128, 128], f8, tag="Yc")
                nc.sync.dma_start(Xc[:], mp_d[r0:r0 + 128, :])
                nc.sync.dma_start(Yc[:], mt_d[r0:r0 + 128, :])
                pg = gpsp.tile([128, 512], f32, tag="pg")
                nc.tensor.matmul(pg[:, 0:128], Xc[:], eye[:], start=True, stop=True)
                nc.tensor.matmul(pg[:, 128:256], Yc[:], eye[:], start=True, stop=True)
                nc.scalar.activation(
                    XTN[:, r0:r0 + 128], pg[:, 0:128], AF.Copy, scale=-2.0
                )
                nc.scalar.activation(YT[:, r0:r0 + 128], pg[:, 128:256], AF.Copy)
                sqA = sqp.tile([128, 128], f32, tag="sqA")
                sqB = sqp.tile([128, 128], f32, tag="sqB")
                nc.scalar.activation(sqA[:], pg[:, 0:128], AF.Square)
                nc.scalar.activation(sqB[:], pg[:, 128:256], AF.Square)
                nc.tensor.matmul(
                    pg[0:1, 256:384], ones128[:], sqA[:], start=True, stop=True
                )
                nc.tensor.matmul(
                    pg[0:1, 384:512], ones128[:], sqB[:], start=True, stop=True
                )
                nc.scalar.activation(xq[0:1, r0:r0 + 128], pg[0:1, 256:384], AF.Copy)
                yst = xcp.tile([1, 128], bf16, tag="yst")
                nc.scalar.activation(yst[:], pg[0:1, 384:512], AF.Copy)
                nc.gpsimd.dma_start(yq[1:2, r0:r0 + 128], yst[:])

            # ---------------- d production ------------------------------
            # block (ib, jc): DP rows [128ib, 128ib+128) x cols [512jc, +512)
            def emit_mm(pg, ib, jc, sl):
                i0, a = 128 * ib, 512 * jc + 128 * sl
                nc.tensor.matmul(
                    pg[:, 128 * sl:128 * sl + 128],
                    XTN[:, i0:i0 + 128], YT[:, a:a + 128],
                    start=True, stop=False,
                )
                nc.tensor.matmul(
                    pg[:, 128 * sl:128 * sl + 128],
                    xq[:, i0:i0 + 128], yq[:, a:a + 128],
                    start=False, stop=True,
                )

            def emit_sqrt(pg, st, sl):
                gap = bass.AP(
                    st.tensor, BW * 8 * sl + 1,
                    [[32 * BW, 128], [BW, 8], [1, W]],
                )
                nc.scalar.activation(gap, pg[:, 128 * sl:128 * sl + 128], AF.Sqrt)

            def emit_dma(st, ib, jc, h):
                # 8 per-chunk DMAs (BIR APs cannot step partitions off-pitch);
                # issued from the Pool queue whose DMA dispatch is ~20x
                # cheaper than SP's
                i0 = 128 * ib
                for cl in range(8 * h, 8 * h + 8):
                    c = 32 * jc + cl
                    src = st[:, cl * BW:(cl + 1) * BW]
                    dst = bass.AP(
                        dsk.tensor,
                        c * DSKC + (i0 + SL * c + SL) * BW,
                        [[DSKC, 1], [BW, 128], [1, BW]],
                    )
                    nc.gpsimd.dma_start(dst, src)

            def emit_block(ib, jc):
                pg = gpsp.tile([128, 512], f32, tag="pg")
                st = stg_tiles[0]
                stg_tiles.reverse()
                for sl in range(4):
                    emit_mm(pg, ib, jc, sl)
                for sl in range(4):
                    emit_sqrt(pg, st, sl)
                for h in range(4):
                    emit_dma(st, ib, jc, h)

            # first 3 i-blocks fully in the prologue (needed from step 0)
            for ib in range(3):
                for jc in range(4):
                    emit_block(ib, jc)

            # remaining 52 blocks paced into the wavefront, deadline order.
            # each block = 10 slots (4 mm-pairs, 4 sqrts, 2 dma) 3 steps apart
            rest = sorted(
                [(ib, jc) for ib in range(3, S // 128) for jc in range(4)],
                key=lambda b: 128 * b[0] + 96 * b[1],
            )
            sched = {}
            blk_state = {}
            for n, (ib, jc) in enumerate(rest):
                base = 10 + 32 * n
                for s in range(4):
                    sched.setdefault(base + 3 * s, []).append(("mm", ib, jc, s))
                for s in range(4):
                    sched.setdefault(base + 12 + 3 * s, []).append(("sq", ib, jc, s))
                for h in range(4):
                    sched.setdefault(base + 22 + 2 * h, []).append(("dm", ib, jc, h))

            # MAE chunk k: prefetch DMA at E_k, DVE consume at E_k + 220
            mpv = mp_d.rearrange("(p x) f -> p (x f)", p=128)
            mtv = mt_d.rearrange("(p x) f -> p (x f)", p=128)
            mae_tiles = {}
            for k in range(n_mch):
                sched.setdefault(120 + 260 * k, []).append(("ml", k))
                sched.setdefault(340 + 260 * k, []).append(("mc", k))

            def emit_sched(t):
                for op in sched.pop(t, ()):
                    if op[0] == "mm":
                        _, ib, jc, sl = op
                        if sl == 0:
                            pg = gpsp.tile([128, 512], f32, tag="pg")
                            st = stg_tiles[0]
                            stg_tiles.reverse()
                            blk_state[(ib, jc)] = (pg, st)
                        pg, st = blk_state[(ib, jc)]
                        emit_mm(pg, ib, jc, sl)
                    elif op[0] == "sq":
                        _, ib, jc, sl = op
                        pg, st = blk_state[(ib, jc)]
                        emit_sqrt(pg, st, sl)
                    elif op[0] == "dm":
                        _, ib, jc, h = op
                        pg, st = blk_state[(ib, jc)]
                        emit_dma(st, ib, jc, h)
                    elif op[0] == "ml":
                        k = op[1]
                        ta = mchp.tile([128, MCH], f8, tag="ma")
                        tb = mchp.tile([128, MCH], f8, tag="mb")
                        nc.sync.dma_start(ta[:], mpv[:, k * MCH:(k + 1) * MCH])
                        nc.sync.dma_start(tb[:], mtv[:, k * MCH:(k + 1) * MCH])
                        mae_tiles[k] = (ta, tb)
                    else:  # "mc"
                        k = op[1]
                        ta, tb = mae_tiles.pop(k)
                        df = dfp.tile([128, MCH], f32, tag="df")
                        nc.vector.tensor_tensor(df[:], ta[:], tb[:], AL.subtract)
                        red = redp.tile([128, 1], f32, tag="red")
                        nc.vector.tensor_reduce(
                            red[:], df[:], mybir.AxisListType.X, AL.add,
                            apply_absolute_value=True,
                        )
                        nc.vector.tensor_tensor(macc[:], macc[:], red[:], AL.add)

            # ---------------- wavefront ---------------------------------
            cur_ps = None
            for t in range(NSTEP):
                if t % 2 == 0:
                    # seeds for steps t, t+1: pst[c] = BIG@c0 + D-col16 of
                    # blocks (t-3),(t-2) shifted down one partition
                    cur_ps = spsp.tile([128, 2], f32, tag="pst")
                    nc.tensor.matmul(
                        cur_ps[:, 0:2], brow[:, 0:128], otwo[:],
                        start=True, stop=False,
                    )
                    nc.tensor.matmul(
                        cur_ps[:, 0:2], shT[:, 0:128],
                        dsk[:, BW * t + W:BW * t + W + 2 * BW:BW],
                        start=False, stop=True,
                    )
                emit_sched(t)
                o = BW * (t + SL)
                prev = dsk[:, o - BW:o]
                nc.vector.tensor_tensor(
                    mB[:, 1:BW], prev[:, 0:W], prev[:, 1:BW], AL.min
                )
                for _ in range(_SCAN_REPS):
                    nc.vector.tensor_tensor_scan(
                        dsk[:, o:o + BW], mB[:, 0:BW], dsk[:, o:o + BW],
                        cur_ps[:, (t % 2):(t % 2) + 1], AL.min, AL.add,
                    )
            # leftover sched events (none expected, but don't drop any)
            for t in sorted(list(sched.keys())):
                emit_sched(t)

            # ---------------- outputs -----------------------------------
            nc.sync.dma_start(dtw_d, dsk[NB - 1:NB, DSKC - 1:DSKC])
            pm = gpsp.tile([128, 512], f32, tag="pg")
            nc.tensor.matmul(
                pm[0:1, 0:1], macc[:, 0:1], ones128[:, 0:1],
                start=True, stop=True,
            )
            nc.scalar.copy(mres[0:1, 0:1], pm[0:1, 0:1])
            nc.sync.dma_start(mae_d, mres[0:1, 0:1])

    return nc


# ---------------------------------------------------------------- runtime
_CACHE = {}


def _get_nc(S, F, mae_rows):
    key = (S, F, mae_rows)
    if key not in _CACHE:
        _CACHE[key] = _build_nc(S, F, mae_rows)
    return _CACHE[key]


class _Runner:
    """Runs nc on 8 cores.  First call goes through run_bass_kernel_spmd
    (compile + validate); later calls reuse a cached compiled executable of
    the identical shard_map computation, skipping the ~1s re-trace/re-lower
    that run_bass_kernel_spmd pays per call."""

    def __init__(self, nc, n_cores=8):
        import concourse.mybir as mybir

        self.nc = nc
        self.n_cores = n_cores
        self.compiled = None
        self.ran_spmd = False

        part = nc.partition_id_tensor.name if nc.partition_id_tensor else None
        self.partition_name = part
        in_names, out_names, out_shapes, out_dtypes = [], [], [], []
        for alloc in nc.m.functions[0].allocations:
            if not isinstance(alloc, mybir.MemoryLocationSet):
                continue
            name = alloc.memorylocations[0].name
            if alloc.kind == "ExternalInput":
                if name != part:
                    in_names.append(name)
            elif alloc.kind == "ExternalOutput":
                out_names.append(name)
                out_shapes.append(tuple(alloc.tensor_shape))
                out_dtypes.append(mybir.dt.np(alloc.dtype))
        self.in_names = in_names
        self.out_names = out_names
        self.out_shapes = out_shapes
        self.out_dtypes = out_dtypes

    def _build_compiled(self, concat_in):
        import jax
        import numpy as np
        from jax.sharding import Mesh, PartitionSpec
        from jax.experimental.shard_map import shard_map
        from concourse.bass2jax import _bass_exec_p, install_neuronx_cc_hook

        install_neuronx_cc_hook()
        nc, n_cores = self.nc, self.n_cores
        out_avals = tuple(
            jax.core.ShapedArray(s, d)
            for s, d in zip(self.out_shapes, self.out_dtypes)
        )
        all_names = list(self.in_names) + list(self.out_names)
        if self.partition_name is not None:
            all_names.append(self.partition_name)
        n_params, n_outs = len(self.in_names), len(self.out_names)
        out_names = tuple(self.out_names)
        partition_name = self.partition_name

        def _body(*args):
            operands = list(args)
            if partition_name is not None:
                from concourse.bass2jax import partition_id_tensor

                operands.append(partition_id_tensor())
            outs = _bass_exec_p.bind(
                *operands,
                out_avals=out_avals,
                in_names=tuple(all_names),
                out_names=out_names,
                lowering_input_output_aliases=(),
                sim_require_finite=True,
                sim_require_nnan=True,
                nc=nc,
            )
            return tuple(outs)

        devices = jax.devices()[:n_cores]
        mesh = Mesh(np.asarray(devices), ("core",))
        in_specs = (PartitionSpec("core"),) * (n_params + n_outs)
        out_specs = (PartitionSpec("core"),) * n_outs
        donate = tuple(range(n_params, n_params + n_outs))
        sharded = jax.jit(
            shard_map(
                _body, mesh=mesh, in_specs=in_specs, out_specs=out_specs,
                check_rep=False,
            ),
            donate_argnums=donate,
            keep_unused=True,
        )
        lowered = sharded.lower(*concat_in, *self._concat_zeros())
        self.compiled = lowered.compile()

    def _concat_zeros(self):
        return [
            np.zeros((self.n_cores * s[0], *s[1:]), d)
            for s, d in zip(self.out_shapes, self.out_dtypes)
        ]

    def _run_fast(self, concat_in):
        import jax
        from jax.sharding import Mesh, NamedSharding, PartitionSpec

        if self.compiled is None:
            self._build_compiled(concat_in)
        # explicit device_put is ~70ms cheaper than letting the executable
        # dispatch convert host ndarrays itself
        mesh = Mesh(np.asarray(jax.devices()[:self.n_cores]), ("core",))
        sh = NamedSharding(mesh, PartitionSpec("core"))
        dev_in = [jax.device_put(a, sh) for a in concat_in]
        dev_z = [jax.device_put(z, sh) for z in self._concat_zeros()]
        out_arrs = self.compiled(*dev_in, *dev_z)
        jax.block_until_ready(out_arrs)
        return [
            {
                name: np.asarray(out_arrs[i]).reshape(
                    self.n_cores, *self.out_shapes[i]
                )[c]
                for i, name in enumerate(self.out_names)
            }
            for c in range(self.n_cores)
        ]

    def run(self, concat_in):
        """concat_in: arrays in self.in_names order, axis-0-concatenated
        over cores.  Returns per-core dicts of outputs."""
        if not self.ran_spmd:
            # first call: the documented compile+run path (also warms the
            # NEFF cache for the cached fast path, which reuses the same
            # backend compile).
            from concourse.bass_utils import run_bass_kernel_spmd

            shard0 = [a.shape[0] // self.n_cores for a in concat_in]
            in_maps = [
                {
                    name: concat_in[i][c * shard0[i]:(c + 1) * shard0[i]]
                    for i, name in enumerate(self.in_names)
                }
                for c in range(self.n_cores)
            ]
            res = run_bass_kernel_spmd(
                self.nc, in_maps, core_ids=list(range(self.n_cores))
            )
            self.ran_spmd = True
            return res.results
        try:
            return self._run_fast(concat_in)
        except Exception:
            from concourse.bass_utils import run_bass_kernel_spmd

            shard0 = [a.shape[0] // self.n_cores for a in concat_in]
            in_maps = [
                {
                    name: concat_in[i][c * shard0[i]:(c + 1) * shard0[i]]
                    for i, name in enumerate(self.in_names)
                }
                for c in range(self.n_cores)
            ]
            res = run_bass_kernel_spmd(
                self.nc, in_maps, core_ids=list(range(self.n_cores))
            )
            return res.results


def _get_runner(S, F, mae_rows):
    key = ("runner", S, F, mae_rows)
    if key not in _CACHE:
        _CACHE[key] = _Runner(_get_nc(S, F, mae_rows))
    return _CACHE[key]


_FP8_LUT = None


def _to_fp8_bits(x32):
    """f32 -> fp8e4m3 via RTNE-to-bf16 then a 64K LUT (2x faster than
    ml_dtypes' direct cast; the double rounding moves only exact-halfway
    cases by <=1 fp8 ulp)."""
    import ml_dtypes

    global _FP8_LUT
    if _FP8_LUT is None:
        _FP8_LUT = (
            np.arange(65536, dtype=np.uint16)
            .view(ml_dtypes.bfloat16)
            .astype(ml_dtypes.float8_e4m3)
            .view(np.uint8)
        )
    return _FP8_LUT[x32.astype(ml_dtypes.bfloat16).view(np.uint16)]


def _prepare(pred, target):
    """Host-side input prep: one fp8 tensor whose per-core axis-0 slice is
    [pred shard (4096 rows); target shard (4096 rows)]."""
    import ml_dtypes

    B, S, F = pred.shape
    n_cores = 8
    shard = B // n_cores
    mae_rows = shard * S
    pred = np.ascontiguousarray(pred, dtype=np.float32)
    target = np.ascontiguousarray(target, dtype=np.float32)
    md = np.empty((n_cores, 2 * mae_rows, F), np.uint8)
    md[:, :mae_rows] = _to_fp8_bits(pred.reshape(n_cores, mae_rows, F))
    md[:, mae_rows:] = _to_fp8_bits(target.reshape(n_cores, mae_rows, F))
    md_all = md.reshape(n_cores * 2 * mae_rows, F).view(ml_dtypes.float8_e4m3)
    return md_all, mae_rows


def kernel(pred, target):
    pred = np.asarray(pred)
    target = np.asarray(target)
    B, S, F = pred.shape
    md_all, mae_rows = _prepare(pred, target)
    runner = _get_runner(S, F, mae_rows)
    results = runner.run([md_all])
    mae_sum = sum(float(results[k]["mae"][0, 0]) for k in range(8))
    dtw = float(results[0]["dtw"][0, 0])
    loss = 0.5 * (mae_sum / (B * S * F)) + 0.5 * (dtw / (S * F))
    return np.float32(loss)


# revision 14
# speedup vs baseline: 6.5344x; 1.2296x over previous
"""DTW loss (0.5*MAE + 0.5*DTW(pred[0],target[0])/(S*F)) on 8 TRN2 cores.

v4: same slope-3 anti-diagonal wavefront DP as v3, but the host->device
contract is slimmed to the information-theoretic minimum and the runtime
path is cached:

* Only two inputs are shipped per core: the bf16 batch shards mp/mt
  ([4096,128] each, 2MB/core).  Everything the DTW needs -- the F-major
  transposes XTN=-2*x^T / YT=y^T, the squared-norm rows xq/yq, the
  shift/identity matrices, poison rows -- is derived ON DEVICE in the
  prologue (PE transposes via identity matmul, ACT Square + ones-matmul
  for the norms, diagonal DMAs for I / shift).  This cuts the axon
  host->device transfer from 42.6MB to 16.8MB (~50MB/s tunnel).

* run_bass_kernel_spmd re-traces + re-lowers a fresh jax.jit on every
  call (~1s of host overhead).  kernel() uses it once (first call:
  compile + validate), then switches to a cached compiled executable of
  the identical shard_map computation, so steady-state calls pay only
  input transfer + execution.

The [S,S] DP is split into NB=128 column chunks of W=16, chunk c in SBUF
partition c.  At wavefront step t partition c processes DP row i = t - 3*c.
The whole row recurrence D[i,j] = d[i,j] + min(D[i-1,j-1], D[i-1,j], D[i,j-1])
is ONE hardware TensorTensorScan per step:

    state = seed;  state = (m[j] min state) add d[j]

with m[j] = min(D[i-1,j-1], D[i-1,j]) (one pair-min tensor_tensor) and
seed = D[i, 16c-1] from the left-neighbour chunk via a tiny PE shift-matmul
(batched two steps per matmul, 3 steps of slack so PE stays off the critical
path).  So the serial chain is just 2 DVE instructions per step.

Storage is a 17-column-per-step in-place ring: block(t) = dsk[:, 17*(t+3)+0:17]
holds [halo, d0..d15] and the scan overwrites it with [halo, D0..D15]
(out == data1; col0 has d=0 so out[0] = seed, materializing next step's halo).
Out-of-range regions are poisoned BIG so no masking is needed.

d[i,j] = sqrt(|x_i|^2+|y_j|^2-2 x.y) is produced on the fly: bf16 GEMM with
the squared norms folded in as a rank-2 augmented matmul -> ACT sqrt (written
with a 17-stride gapped AP) -> block-fused skew DMAs, all paced into the
wavefront's idle engine time.

The MAE term is data-parallel over the batch; each core reduces its 2-sample
shard on DVE in 8 chunks interleaved into the wavefront (DMA-prefetched).
"""

import numpy as np

_BIG = 1.0e30
_S, _F, _B = 2048, 128, 16
_W = 16
_BW = 17
_SLOPE = 3
_SCAN_REPS = 1  # >1 only for timing experiments (corrupts the DP)


# ---------------------------------------------------------------- patches
def _apply_walrus_patches():
    """This walrus build rejects >1 semaphore wait per instruction; peel
    extra waits onto same-engine Drain instructions."""
    import bass_rust
    import concourse.mybir as mybir
    from concourse import tile
    from concourse.vector_clock import ScopedClock

    if getattr(tile.TileContext, "_wait_split_patched", False):
        return
    _orig_add = tile.TileContext._add_instruction

    def _mk_drain(nc, engine, waits):
        d = mybir.InstDrain(name=nc.get_next_instruction_name(), engine=engine)
        d.sync_info = bass_rust.SyncInfo(on_wait=list(waits), on_update=[])
        return d

    def _add_split(self, inst):
        si = inst.sync_info
        if (
            si is not None
            and si.on_wait
            and len(si.on_wait) > 1
            and inst.engine is not None
            and inst.engine != mybir.EngineType.Unassigned
        ):
            waits = list(si.on_wait)
            si.on_wait = waits[:1]
            for w in waits[1:]:
                _orig_add(self, _mk_drain(self.nc, inst.engine, [w]))
        _orig_add(self, inst)

    def _drain_and_barrier_split(self, tick_clock, wait_clock):
        nc = self.nc
        drain_inst = nc.sync.drain()
        wait_clock.add_sem_waits(
            drain_inst.ins, ScopedClock({None: tick_clock.global_clock})
        )
        si = drain_inst.ins.sync_info
        waits = list(si.on_wait) if si and si.on_wait else []
        if len(waits) > 1:
            si.on_wait = waits[:1]
            for w in waits[1:]:
                d2 = nc.sync.drain()
                d2.ins.sync_info = bass_rust.SyncInfo(on_wait=[w], on_update=[])
        nc.all_engine_barrier()
        assert self.sems is not None
        popped = nc._tile_sem_poison_stack.pop()
        assert popped is self._sem_poison
        nc.clear_and_free_semaphores(list(self.sems.allocated().values()))
        nc.all_engine_barrier()

    tile.TileContext._add_instruction = _add_split
    tile.TileContext._drain_and_barrier = _drain_and_barrier_split
    tile.TileContext._wait_split_patched = True


# ---------------------------------------------------------------- builder
def _build_nc(S, F, mae_rows):
    import concourse.bass as bass
    import concourse.mybir as mybir
    from concourse import tile

    _apply_walrus_patches()

    f32 = mybir.dt.float32
    bf16 = mybir.dt.bfloat16
    f8 = mybir.dt.float8e4
    AL = mybir.AluOpType
    AF = mybir.ActivationFunctionType

    W, BW, SL = _W, _BW, _SLOPE
    NB = S // W                          # 128 column chunks
    NSTEP = S + SL * (NB - 1)            # 2429 wavefront steps
    NBLK = NSTEP + SL                    # absolute 17-col blocks
    DSKC = BW * NBLK                     # dsk free cols (41344)
    HEADC = BW * (SL * NB)               # head poison cols [0, 6528)
    TAILC = BW * (SL + S)                # tail poison from col 34867
    NCH = S // 128                       # 16 derivation chunks

    nc = bass.Bass("TRN2", target_bir_lowering=False, debug=False, num_devices=8)

    # one fp8 input: rows [0,mae_rows) = pred shard, [mae_rows,2m) = target
    md_d = nc.dram_tensor("md", [2 * mae_rows, F], f8, kind="ExternalInput").ap()
    mp_d = md_d[0:mae_rows, :]
    mt_d = md_d[mae_rows:2 * mae_rows, :]
    out_d = nc.dram_tensor("out", [1, 2], f32, kind="ExternalOutput").ap()

    mae_free = mae_rows * F // 128       # 4096 cols per partition
    MCH = 512
    n_mch = mae_free // MCH              # 8 chunks

    with tile.TileContext(nc) as tc:
        with (
            tc.tile_pool(name="big", bufs=1) as bp,
            tc.tile_pool(name="stg", bufs=2) as stgp,
            tc.tile_pool(name="mch", bufs=4) as mchp,
            tc.tile_pool(name="dfp", bufs=2) as dfp,
            tc.tile_pool(name="red", bufs=2) as redp,
            tc.tile_pool(name="xcp", bufs=4) as xcp,
            tc.tile_pool(name="sqp", bufs=4) as sqp,
            tc.tile_pool(name="gps", bufs=2, space=bass.MemorySpace.PSUM) as gpsp,
            tc.tile_pool(name="sps", bufs=4, space=bass.MemorySpace.PSUM) as spsp,
        ):
            dsk = bp.tile([128, DSKC], f32, tag="dsk")
            XTN = bp.tile([128, S], bf16, tag="XTN")
            YT = bp.tile([128, S], bf16, tag="YT")
            xq = bp.tile([2, S], bf16, tag="xq")
            yq = bp.tile([2, S], bf16, tag="yq")
            mB = bp.tile([128, BW], f32, tag="mB")
            shT = bp.tile([128, 128], f32, tag="shT")
            eye = bp.tile([128, 128], f8, tag="eye")
            brow = bp.tile([1, 128], f32, tag="brow")
            otwo = bp.tile([1, 2], f32, tag="otwo")
            ones128 = bp.tile([128, 1], f32, tag="ones128")
            macc = bp.tile([128, 1], f32, tag="macc")
            mres = bp.tile([1, 1], f32, tag="mres")

            # ---------------- prologue: consts + poison -----------------
            nc.gpsimd.memset(ones128[:], 1.0)
            nc.gpsimd.memset(macc[:], 0.0)
            nc.gpsimd.memset(mB[:, 0:1], _BIG)
            nc.gpsimd.memset(brow[:], 0.0)
            nc.gpsimd.memset(brow[0:1, 0:1], _BIG)
            nc.gpsimd.memset(otwo[:], 1.0)
            nc.gpsimd.memset(eye[:], 1.0)
            nc.gpsimd.memset(shT[:], 1.0)
            # engine writes must start at partition 0: fill both rows with
            # ones, then overwrite the norm row (xq row0 via ACT; yq row1 is
            # at partition 1, so it goes through a partition-0 staging tile
            # + DMA, which has no partition-start restriction)
            nc.gpsimd.memset(xq[:, :], 1.0)
            nc.gpsimd.memset(yq[:, :], 1.0)
            # eye[p,j] = (j-p == 0), shT[p,j] = (j-p-1 == 0)
            nc.gpsimd.affine_select(
                eye[:], eye[:], [[1, 128]], AL.is_equal, 0.0,
                base=0, channel_multiplier=-1,
            )
            nc.gpsimd.affine_select(
                shT[:], shT[:], [[1, 128]], AL.is_equal, 0.0,
                base=-1, channel_multiplier=-1,
            )
            nc.vector.memset(dsk[:, 0:HEADC], _BIG)
            nc.gpsimd.memset(dsk[:, TAILC:DSKC], _BIG)
            # DP start cell: block(-1)[chunk0, col0] = 0 enables D[0,0]=d[0,0]
            nc.gpsimd.memset(dsk[0:1, (SL - 1) * BW:(SL - 1) * BW + 1], 0.0)

            # zero col0 of all 32 chunk-groups in both stg buffers (the skew
            # DMA carries them into block col0 = the scan's "d=0" halo slot)
            stg_tiles = []
            for _ in range(2):
                s = stgp.tile([128, 32 * BW], f32, tag="stg")
                for cl in range(32):
                    nc.gpsimd.memset(s[:, cl * BW:cl * BW + 1], 0.0)
                stg_tiles.append(s)

            # ---------------- on-device DTW input derivation ------------
            # x = rows [0,2048) of this core's mp shard (= pred[0] on core 0),
            # y = rows [0,2048) of mt.  Per 128-row chunk c, one PSUM bank:
            #   PE: pg[:,0:128] = Xc^T, pg[:,128:256] = Yc^T (identity mm)
            #   ACT: XTN[:,c] = -2*Xc^T (bf16), YT[:,c] = Yc^T (bf16),
            #        sqA/sqB = Square(transposes) (f32 SBUF)
            #   PE: pg[0,256:384] = ones^T @ sqA = |x_s|^2 row (same for y)
            #   ACT: xq[0, c*128:+128] / yq[1, ...] = norm rows (bf16)
            for c in range(NCH):
                r0 = 128 * c
                Xc = xcp.tile([128, 128], f8, tag="Xc")
                Yc = xcp.tile([128, 128], f8, tag="Yc")
                nc.sync.dma_start(Xc[:], mp_d[r0:r0 + 128, :])
                nc.sync.dma_start(Yc[:], mt_d[r0:r0 + 128, :])
                pg = gpsp.tile([128, 512], f32, tag="pg")
                nc.tensor.matmul(pg[:, 0:128], Xc[:], eye[:], start=True, stop=True)
                nc.tensor.matmul(pg[:, 128:256], Yc[:], eye[:], start=True, stop=True)
                nc.scalar.activation(
                    XTN[:, r0:r0 + 128], pg[:, 0:128], AF.Copy, scale=-2.0
                )
                nc.scalar.activation(YT[:, r0:r0 + 128], pg[:, 128:256], AF.Copy)
                sqA = sqp.tile([128, 128], f32, tag="sqA")
                sqB = sqp.tile([128, 128], f32, tag="sqB")
                nc.scalar.activation(sqA[:], pg[:, 0:128], AF.Square)
                nc.scalar.activation(sqB[:], pg[:, 128:256], AF.Square)
                nc.tensor.matmul(
                    pg[0:1, 256:384], ones128[:], sqA[:], start=True, stop=True
                )
                nc.tensor.matmul(
                    pg[0:1, 384:512], ones128[:], sqB[:], start=True, stop=True
                )
                nc.scalar.activation(xq[0:1, r0:r0 + 128], pg[0:1, 256:384], AF.Copy)
                yst = xcp.tile([1, 128], bf16, tag="yst")
                nc.scalar.activation(yst[:], pg[0:1, 384:512], AF.Copy)
                nc.gpsimd.dma_start(yq[1:2, r0:r0 + 128], yst[:])

            # ---------------- d production ------------------------------
            # block (ib, jc): DP rows [128ib, 128ib+128) x cols [512jc, +512)
            def emit_mm(pg, ib, jc, sl):
                i0, a = 128 * ib, 512 * jc + 128 * sl
                nc.tensor.matmul(
                    pg[:, 128 * sl:128 * sl + 128],
                    XTN[:, i0:i0 + 128], YT[:, a:a + 128],
                    start=True, stop=False,
                )
                nc.tensor.matmul(
                    pg[:, 128 * sl:128 * sl + 128],
                    xq[:, i0:i0 + 128], yq[:, a:a + 128],
                    start=False, stop=True,
                )

            def emit_sqrt(pg, st, sl):
                gap = bass.AP(
                    st.tensor, BW * 8 * sl + 1,
                    [[32 * BW, 128], [BW, 8], [1, W]],
                )
                nc.scalar.activation(gap, pg[:, 128 * sl:128 * sl + 128], AF.Sqrt)

            def emit_dma(st, ib, jc, h):
                # 8 per-chunk DMAs (BIR APs cannot step partitions off-pitch);
                # issued from the Pool queue whose DMA dispatch is ~20x
                # cheaper than SP's
                i0 = 128 * ib
                for cl in range(8 * h, 8 * h + 8):
                    c = 32 * jc + cl
                    src = st[:, cl * BW:(cl + 1) * BW]
                    dst = bass.AP(
                        dsk.tensor,
                        c * DSKC + (i0 + SL * c + SL) * BW,
                        [[DSKC, 1], [BW, 128], [1, BW]],
                    )
                    nc.gpsimd.dma_start(dst, src)

            def emit_block(ib, jc):
                pg = gpsp.tile([128, 512], f32, tag="pg")
                st = stg_tiles[0]
                stg_tiles.reverse()
                for sl in range(4):
                    emit_mm(pg, ib, jc, sl)
                for sl in range(4):
                    emit_sqrt(pg, st, sl)
                for h in range(4):
                    emit_dma(st, ib, jc, h)

            # first 3 i-blocks fully in the prologue (needed from step 0)
            for ib in range(3):
                for jc in range(4):
                    emit_block(ib, jc)

            # remaining 52 blocks paced into the wavefront, deadline order.
            # each block = 10 slots (4 mm-pairs, 4 sqrts, 2 dma) 3 steps apart
            rest = sorted(
                [(ib, jc) for ib in range(3, S // 128) for jc in range(4)],
                key=lambda b: 128 * b[0] + 96 * b[1],
            )
            sched = {}
            blk_state = {}
            for n, (ib, jc) in enumerate(rest):
                base = 10 + 32 * n
                for s in range(4):
                    sched.setdefault(base + 3 * s, []).append(("mm", ib, jc, s))
                for s in range(4):
                    sched.setdefault(base + 12 + 3 * s, []).append(("sq", ib, jc, s))
                for h in range(4):
                    sched.setdefault(base + 22 + 2 * h, []).append(("dm", ib, jc, h))

            # MAE chunk k: prefetch DMA at E_k, DVE consume at E_k + 220
            mpv = mp_d.rearrange("(p x) f -> p (x f)", p=128)
            mtv = mt_d.rearrange("(p x) f -> p (x f)", p=128)
            mae_tiles = {}
            for k in range(n_mch):
                sched.setdefault(120 + 260 * k, []).append(("ml", k))
                sched.setdefault(340 + 260 * k, []).append(("mc", k))

            def emit_sched(t):
                for op in sched.pop(t, ()):
                    if op[0] == "mm":
                        _, ib, jc, sl = op
                        if sl == 0:
                            pg = gpsp.tile([128, 512], f32, tag="pg")
                            st = stg_tiles[0]
                            stg_tiles.reverse()
                            blk_state[(ib, jc)] = (pg, st)
                        pg, st = blk_state[(ib, jc)]
                        emit_mm(pg, ib, jc, sl)
                    elif op[0] == "sq":
                        _, ib, jc, sl = op
                        pg, st = blk_state[(ib, jc)]
                        emit_sqrt(pg, st, sl)
                    elif op[0] == "dm":
                        _, ib, jc, h = op
                        pg, st = blk_state[(ib, jc)]
                        emit_dma(st, ib, jc, h)
                    elif op[0] == "ml":
                        k = op[1]
                        ta = mchp.tile([128, MCH], f8, tag="ma")
                        tb = mchp.tile([128, MCH], f8, tag="mb")
                        nc.sync.dma_start(ta[:], mpv[:, k * MCH:(k + 1) * MCH])
                        nc.sync.dma_start(tb[:], mtv[:, k * MCH:(k + 1) * MCH])
                        mae_tiles[k] = (ta, tb)
                    else:  # "mc"
                        k = op[1]
                        ta, tb = mae_tiles.pop(k)
                        df = dfp.tile([128, MCH], f32, tag="df")
                        nc.vector.tensor_tensor(df[:], ta[:], tb[:], AL.subtract)
                        red = redp.tile([128, 1], f32, tag="red")
                        nc.vector.tensor_reduce(
                            red[:], df[:], mybir.AxisListType.X, AL.add,
                            apply_absolute_value=True,
                        )
                        nc.vector.tensor_tensor(macc[:], macc[:], red[:], AL.add)

            # ---------------- wavefront ---------------------------------
            cur_ps = None
            for t in range(NSTEP):
                if t % 2 == 0:
                    # seeds for steps t, t+1: pst[c] = BIG@c0 + D-col16 of
                    # blocks (t-3),(t-2) shifted down one partition
                    cur_ps = spsp.tile([128, 2], f32, tag="pst")
                    nc.tensor.matmul(
                        cur_ps[:, 0:2], brow[:, 0:128], otwo[:],
                        start=True, stop=False,
                    )
                    nc.tensor.matmul(
                        cur_ps[:, 0:2], shT[:, 0:128],
                        dsk[:, BW * t + W:BW * t + W + 2 * BW:BW],
                        start=False, stop=True,
                    )
                emit_sched(t)
                o = BW * (t + SL)
                prev = dsk[:, o - BW:o]
                nc.vector.tensor_tensor(
                    mB[:, 1:BW], prev[:, 0:W], prev[:, 1:BW], AL.min
                )
                for _ in range(_SCAN_REPS):
                    nc.vector.tensor_tensor_scan(
                        dsk[:, o:o + BW], mB[:, 0:BW], dsk[:, o:o + BW],
                        cur_ps[:, (t % 2):(t % 2) + 1], AL.min, AL.add,
                    )
            # leftover sched events (none expected, but don't drop any)
            for t in sorted(list(sched.keys())):
                emit_sched(t)

            # ---------------- outputs -----------------------------------
            nc.sync.dma_start(out_d[0:1, 0:1], dsk[NB - 1:NB, DSKC - 1:DSKC])
            pm = gpsp.tile([128, 512], f32, tag="pg")
            nc.tensor.matmul(
                pm[0:1, 0:1], macc[:, 0:1], ones128[:, 0:1],
                start=True, stop=True,
            )
            nc.scalar.copy(mres[0:1, 0:1], pm[0:1, 0:1])
            nc.sync.dma_start(out_d[0:1, 1:2], mres[0:1, 0:1])

    return nc


# ---------------------------------------------------------------- runtime
_CACHE = {}


def _get_nc(S, F, mae_rows):
    key = (S, F, mae_rows)
    if key not in _CACHE:
        _CACHE[key] = _build_nc(S, F, mae_rows)
    return _CACHE[key]


class _Runner:
    """Runs nc on 8 cores.  First call goes through run_bass_kernel_spmd
    (compile + validate); later calls reuse a cached compiled executable of
    the identical shard_map computation, skipping the ~1s re-trace/re-lower
    that run_bass_kernel_spmd pays per call."""

    def __init__(self, nc, n_cores=8):
        import concourse.mybir as mybir

        self.nc = nc
        self.n_cores = n_cores
        self.compiled = None
        self.ran_spmd = False

        part = nc.partition_id_tensor.name if nc.partition_id_tensor else None
        self.partition_name = part
        in_names, out_names, out_shapes, out_dtypes = [], [], [], []
        for alloc in nc.m.functions[0].allocations:
            if not isinstance(alloc, mybir.MemoryLocationSet):
                continue
            name = alloc.memorylocations[0].name
            if alloc.kind == "ExternalInput":
                if name != part:
                    in_names.append(name)
            elif alloc.kind == "ExternalOutput":
                out_names.append(name)
                out_shapes.append(tuple(alloc.tensor_shape))
                out_dtypes.append(mybir.dt.np(alloc.dtype))
        self.in_names = in_names
        self.out_names = out_names
        self.out_shapes = out_shapes
        self.out_dtypes = out_dtypes

    def _build_compiled(self, concat_in):
        import jax
        import numpy as np
        from jax.sharding import Mesh, PartitionSpec
        from jax.experimental.shard_map import shard_map
        from concourse.bass2jax import _bass_exec_p, install_neuronx_cc_hook

        install_neuronx_cc_hook()
        nc, n_cores = self.nc, self.n_cores
        out_avals = tuple(
            jax.core.ShapedArray(s, d)
            for s, d in zip(self.out_shapes, self.out_dtypes)
        )
        all_names = list(self.in_names) + list(self.out_names)
        if self.partition_name is not None:
            all_names.append(self.partition_name)
        n_params, n_outs = len(self.in_names), len(self.out_names)
        out_names = tuple(self.out_names)
        partition_name = self.partition_name

        def _body(*args):
            operands = list(args)
            if partition_name is not None:
                from concourse.bass2jax import partition_id_tensor

                operands.append(partition_id_tensor())
            outs = _bass_exec_p.bind(
                *operands,
                out_avals=out_avals,
                in_names=tuple(all_names),
                out_names=out_names,
                lowering_input_output_aliases=(),
                sim_require_finite=True,
                sim_require_nnan=True,
                nc=nc,
            )
            return tuple(outs)

        devices = jax.devices()[:n_cores]
        mesh = Mesh(np.asarray(devices), ("core",))
        in_specs = (PartitionSpec("core"),) * (n_params + n_outs)
        out_specs = (PartitionSpec("core"),) * n_outs
        donate = tuple(range(n_params, n_params + n_outs))
        sharded = jax.jit(
            shard_map(
                _body, mesh=mesh, in_specs=in_specs, out_specs=out_specs,
                check_rep=False,
            ),
            donate_argnums=donate,
            keep_unused=True,
        )
        lowered = sharded.lower(*concat_in, *self._concat_zeros())
        self.compiled = lowered.compile()

    def _concat_zeros(self):
        return [
            np.zeros((self.n_cores * s[0], *s[1:]), d)
            for s, d in zip(self.out_shapes, self.out_dtypes)
        ]

    def _run_fast(self, concat_in):
        import jax
        from jax.sharding import Mesh, NamedSharding, PartitionSpec

        if self.compiled is None:
            self._build_compiled(concat_in)
        # explicit device_put is ~70ms cheaper than letting the executable
        # dispatch convert host ndarrays itself
        mesh = Mesh(np.asarray(jax.devices()[:self.n_cores]), ("core",))
        sh = NamedSharding(mesh, PartitionSpec("core"))
        dev_in = [jax.device_put(a, sh) for a in concat_in]
        out_arrs = self.compiled(*dev_in, *self._concat_zeros())
        jax.block_until_ready(out_arrs)
        return [
            {
                name: np.asarray(out_arrs[i]).reshape(
                    self.n_cores, *self.out_shapes[i]
                )[c]
                for i, name in enumerate(self.out_names)
            }
            for c in range(self.n_cores)
        ]

    def run(self, concat_in):
        """concat_in: arrays in self.in_names order, axis-0-concatenated
        over cores.  Returns per-core dicts of outputs."""
        if not self.ran_spmd:
            # first call: the documented compile+run path (also warms the
            # NEFF cache for the cached fast path, which reuses the same
            # backend compile).
            from concourse.bass_utils import run_bass_kernel_spmd

            shard0 = [a.shape[0] // self.n_cores for a in concat_in]
            in_maps = [
                {
                    name: concat_in[i][c * shard0[i]:(c + 1) * shard0[i]]
                    for i, name in enumerate(self.in_names)
                }
                for c in range(self.n_cores)
            ]
            res = run_bass_kernel_spmd(
                self.nc, in_maps, core_ids=list(range(self.n_cores))
            )
            self.ran_spmd = True
            return res.results
        try:
            return self._run_fast(concat_in)
        except Exception:
            from concourse.bass_utils import run_bass_kernel_spmd

            shard0 = [a.shape[0] // self.n_cores for a in concat_in]
            in_maps = [
                {
                    name: concat_in[i][c * shard0[i]:(c + 1) * shard0[i]]
                    for i, name in enumerate(self.in_names)
                }
                for c in range(self.n_cores)
            ]
            res = run_bass_kernel_spmd(
                self.nc, in_maps, core_ids=list(range(self.n_cores))
            )
            return res.results


def _get_runner(S, F, mae_rows):
    key = ("runner", S, F, mae_rows)
    if key not in _CACHE:
        _CACHE[key] = _Runner(_get_nc(S, F, mae_rows))
    return _CACHE[key]


_FP8_LUT = None


def _to_fp8_bits(x32):
    """f32 -> fp8e4m3 via RTNE-to-bf16 then a 64K LUT (2x faster than
    ml_dtypes' direct cast; the double rounding moves only exact-halfway
    cases by <=1 fp8 ulp)."""
    import ml_dtypes

    global _FP8_LUT
    if _FP8_LUT is None:
        _FP8_LUT = (
            np.arange(65536, dtype=np.uint16)
            .view(ml_dtypes.bfloat16)
            .astype(ml_dtypes.float8_e4m3)
            .view(np.uint8)
        )
    return _FP8_LUT[x32.astype(ml_dtypes.bfloat16).view(np.uint16)]


def _prepare(pred, target):
    """Host-side input prep: one fp8 tensor whose per-core axis-0 slice is
    [pred shard (4096 rows); target shard (4096 rows)]."""
    import ml_dtypes

    B, S, F = pred.shape
    n_cores = 8
    shard = B // n_cores
    mae_rows = shard * S
    pred = np.ascontiguousarray(pred, dtype=np.float32)
    target = np.ascontiguousarray(target, dtype=np.float32)
    md = np.empty((n_cores, 2 * mae_rows, F), np.uint8)
    md[:, :mae_rows] = _to_fp8_bits(pred.reshape(n_cores, mae_rows, F))
    md[:, mae_rows:] = _to_fp8_bits(target.reshape(n_cores, mae_rows, F))
    md_all = md.reshape(n_cores * 2 * mae_rows, F).view(ml_dtypes.float8_e4m3)
    return md_all, mae_rows


def kernel(pred, target):
    pred = np.asarray(pred)
    target = np.asarray(target)
    B, S, F = pred.shape
    md_all, mae_rows = _prepare(pred, target)
    runner = _get_runner(S, F, mae_rows)
    results = runner.run([md_all])
    mae_sum = sum(float(results[k]["out"][0, 1]) for k in range(8))
    dtw = float(results[0]["out"][0, 0])
    loss = 0.5 * (mae_sum / (B * S * F)) + 0.5 * (dtw / (S * F))
    return np.float32(loss)


# revision 15
# speedup vs baseline: 8.2189x; 1.2578x over previous
"""DTW loss (0.5*MAE + 0.5*DTW(pred[0],target[0])/(S*F)) on 8 TRN2 cores.

v4: same slope-3 anti-diagonal wavefront DP as v3, but the host->device
contract is slimmed to the information-theoretic minimum and the runtime
path is cached:

* Only two inputs are shipped per core: the bf16 batch shards mp/mt
  ([4096,128] each, 2MB/core).  Everything the DTW needs -- the F-major
  transposes XTN=-2*x^T / YT=y^T, the squared-norm rows xq/yq, the
  shift/identity matrices, poison rows -- is derived ON DEVICE in the
  prologue (PE transposes via identity matmul, ACT Square + ones-matmul
  for the norms, diagonal DMAs for I / shift).  This cuts the axon
  host->device transfer from 42.6MB to 16.8MB (~50MB/s tunnel).

* run_bass_kernel_spmd re-traces + re-lowers a fresh jax.jit on every
  call (~1s of host overhead).  kernel() uses it once (first call:
  compile + validate), then switches to a cached compiled executable of
  the identical shard_map computation, so steady-state calls pay only
  input transfer + execution.

The [S,S] DP is split into NB=128 column chunks of W=16, chunk c in SBUF
partition c.  At wavefront step t partition c processes DP row i = t - 3*c.
The whole row recurrence D[i,j] = d[i,j] + min(D[i-1,j-1], D[i-1,j], D[i,j-1])
is ONE hardware TensorTensorScan per step:

    state = seed;  state = (m[j] min state) add d[j]

with m[j] = min(D[i-1,j-1], D[i-1,j]) (one pair-min tensor_tensor) and
seed = D[i, 16c-1] from the left-neighbour chunk via a tiny PE shift-matmul
(batched two steps per matmul, 3 steps of slack so PE stays off the critical
path).  So the serial chain is just 2 DVE instructions per step.

Storage is a 17-column-per-step in-place ring: block(t) = dsk[:, 17*(t+3)+0:17]
holds [halo, d0..d15] and the scan overwrites it with [halo, D0..D15]
(out == data1; col0 has d=0 so out[0] = seed, materializing next step's halo).
Out-of-range regions are poisoned BIG so no masking is needed.

d[i,j] = sqrt(|x_i|^2+|y_j|^2-2 x.y) is produced on the fly: bf16 GEMM with
the squared norms folded in as a rank-2 augmented matmul -> ACT sqrt (written
with a 17-stride gapped AP) -> block-fused skew DMAs, all paced into the
wavefront's idle engine time.

The MAE term is data-parallel over the batch; each core reduces its 2-sample
shard on DVE in 8 chunks interleaved into the wavefront (DMA-prefetched).
"""

import numpy as np

_BIG = 1.0e30
_S, _F, _B = 2048, 128, 16
_W = 16
_BW = 17
_SLOPE = 3
_SCAN_REPS = 1  # >1 only for timing experiments (corrupts the DP)


# ---------------------------------------------------------------- patches
def _apply_walrus_patches():
    """This walrus build rejects >1 semaphore wait per instruction; peel
    extra waits onto same-engine Drain instructions."""
    import bass_rust
    import concourse.mybir as mybir
    from concourse import tile
    from concourse.vector_clock import ScopedClock

    if getattr(tile.TileContext, "_wait_split_patched", False):
        return
    _orig_add = tile.TileContext._add_instruction

    def _mk_drain(nc, engine, waits):
        d = mybir.InstDrain(name=nc.get_next_instruction_name(), engine=engine)
        d.sync_info = bass_rust.SyncInfo(on_wait=list(waits), on_update=[])
        return d

    def _add_split(self, inst):
        si = inst.sync_info
        if (
            si is not None
            and si.on_wait
            and len(si.on_wait) > 1
            and inst.engine is not None
            and inst.engine != mybir.EngineType.Unassigned
        ):
            waits = list(si.on_wait)
            si.on_wait = waits[:1]
            for w in waits[1:]:
                _orig_add(self, _mk_drain(self.nc, inst.engine, [w]))
        _orig_add(self, inst)

    def _drain_and_barrier_split(self, tick_clock, wait_clock):
        nc = self.nc
        drain_inst = nc.sync.drain()
        wait_clock.add_sem_waits(
            drain_inst.ins, ScopedClock({None: tick_clock.global_clock})
        )
        si = drain_inst.ins.sync_info
        waits = list(si.on_wait) if si and si.on_wait else []
        if len(waits) > 1:
            si.on_wait = waits[:1]
            for w in waits[1:]:
                d2 = nc.sync.drain()
                d2.ins.sync_info = bass_rust.SyncInfo(on_wait=[w], on_update=[])
        nc.all_engine_barrier()
        assert self.sems is not None
        popped = nc._tile_sem_poison_stack.pop()
        assert popped is self._sem_poison
        nc.clear_and_free_semaphores(list(self.sems.allocated().values()))
        nc.all_engine_barrier()

    tile.TileContext._add_instruction = _add_split
    tile.TileContext._drain_and_barrier = _drain_and_barrier_split
    tile.TileContext._wait_split_patched = True


# ---------------------------------------------------------------- builder
def _build_nc(S, F, mae_rows):
    import concourse.bass as bass
    import concourse.mybir as mybir
    from concourse import tile

    _apply_walrus_patches()

    f32 = mybir.dt.float32
    bf16 = mybir.dt.bfloat16
    f8 = mybir.dt.float8e4
    AL = mybir.AluOpType
    AF = mybir.ActivationFunctionType

    W, BW, SL = _W, _BW, _SLOPE
    NB = S // W                          # 128 column chunks
    NSTEP = S + SL * (NB - 1)            # 2429 wavefront steps
    NBLK = NSTEP + SL                    # absolute 17-col blocks
    DSKC = BW * NBLK                     # dsk free cols (41344)
    HEADC = BW * (SL * NB)               # head poison cols [0, 6528)
    TAILC = BW * (SL + S)                # tail poison from col 34867
    NCH = S // 128                       # 16 derivation chunks

    nc = bass.Bass("TRN2", target_bir_lowering=False, debug=False, num_devices=8)

    # fp8 inputs: dd = this core's (pred - target) shard for the MAE;
    # xy = this core's 1/8 slice of (pred[0]; target[0]) for the DTW, which
    # is AllGathered on device over NeuronLink (ships 0.5MB instead of 8MB)
    dd_d = nc.dram_tensor("dd", [mae_rows, F], f8, kind="ExternalInput").ap()
    xy_d = nc.dram_tensor("xy", [S // 4, F], f8, kind="ExternalInput").ap()
    xys_d = nc.dram_tensor("xys", [S // 4, F], f8).ap()    # collective staging
    gath_d = nc.dram_tensor("gath", [2 * S, F], f8).ap()   # gathered x0;y0
    out_d = nc.dram_tensor("out", [1, 2], f32, kind="ExternalOutput").ap()

    mae_free = mae_rows * F // 128       # 4096 cols per partition
    MCH = 512
    n_mch = mae_free // MCH              # 8 chunks

    with tile.TileContext(nc) as tc:
        with (
            tc.tile_pool(name="big", bufs=1) as bp,
            tc.tile_pool(name="stg", bufs=2) as stgp,
            tc.tile_pool(name="mch", bufs=4) as mchp,
            tc.tile_pool(name="red", bufs=2) as redp,
            tc.tile_pool(name="xcp", bufs=4) as xcp,
            tc.tile_pool(name="sqp", bufs=4) as sqp,
            tc.tile_pool(name="gps", bufs=2, space=bass.MemorySpace.PSUM) as gpsp,
            tc.tile_pool(name="sps", bufs=4, space=bass.MemorySpace.PSUM) as spsp,
        ):
            dsk = bp.tile([128, DSKC], f32, tag="dsk")
            XTN = bp.tile([128, S], bf16, tag="XTN")
            YT = bp.tile([128, S], bf16, tag="YT")
            xq = bp.tile([2, S], bf16, tag="xq")
            yq = bp.tile([2, S], bf16, tag="yq")
            mB = bp.tile([128, BW], f32, tag="mB")
            shT = bp.tile([128, 128], f32, tag="shT")
            eye = bp.tile([128, 128], f8, tag="eye")
            brow = bp.tile([1, 128], f32, tag="brow")
            otwo = bp.tile([1, 2], f32, tag="otwo")
            ones128 = bp.tile([128, 1], f32, tag="ones128")
            macc = bp.tile([128, 1], f32, tag="macc")
            mres = bp.tile([1, 1], f32, tag="mres")

            # ---------------- prologue: gather x0/y0 --------------------
            # collectives cannot read IO tensors: stage input -> internal
            nc.sync.dma_start(xys_d, xy_d)
            nc.gpsimd.collective_compute(
                "AllGather", AL.bypass,
                replica_groups=[[0, 1, 2, 3, 4, 5, 6, 7]],
                ins=[xys_d.opt()], outs=[gath_d.opt()],
            )

            # ---------------- prologue: consts + poison -----------------
            nc.gpsimd.memset(ones128[:], 1.0)
            nc.gpsimd.memset(macc[:], 0.0)
            nc.gpsimd.memset(mB[:, 0:1], _BIG)
            nc.gpsimd.memset(brow[:], 0.0)
            nc.gpsimd.memset(brow[0:1, 0:1], _BIG)
            nc.gpsimd.memset(otwo[:], 1.0)
            nc.gpsimd.memset(eye[:], 1.0)
            nc.gpsimd.memset(shT[:], 1.0)
            # engine writes must start at partition 0: fill both rows with
            # ones, then overwrite the norm row (xq row0 via ACT; yq row1 is
            # at partition 1, so it goes through a partition-0 staging tile
            # + DMA, which has no partition-start restriction)
            nc.gpsimd.memset(xq[:, :], 1.0)
            nc.gpsimd.memset(yq[:, :], 1.0)
            # eye[p,j] = (j-p == 0), shT[p,j] = (j-p-1 == 0)
            nc.gpsimd.affine_select(
                eye[:], eye[:], [[1, 128]], AL.is_equal, 0.0,
                base=0, channel_multiplier=-1,
            )
            nc.gpsimd.affine_select(
                shT[:], shT[:], [[1, 128]], AL.is_equal, 0.0,
                base=-1, channel_multiplier=-1,
            )
            nc.vector.memset(dsk[:, 0:HEADC], _BIG)
            nc.gpsimd.memset(dsk[:, TAILC:DSKC], _BIG)
            # DP start cell: block(-1)[chunk0, col0] = 0 enables D[0,0]=d[0,0]
            nc.gpsimd.memset(dsk[0:1, (SL - 1) * BW:(SL - 1) * BW + 1], 0.0)

            # zero col0 of all 32 chunk-groups in both stg buffers (the skew
            # DMA carries them into block col0 = the scan's "d=0" halo slot)
            stg_tiles = []
            for _ in range(2):
                s = stgp.tile([128, 32 * BW], f32, tag="stg")
                for cl in range(32):
                    nc.gpsimd.memset(s[:, cl * BW:cl * BW + 1], 0.0)
                stg_tiles.append(s)

            # ---------------- on-device DTW input derivation ------------
            # x = rows [0,2048) of this core's mp shard (= pred[0] on core 0),
            # y = rows [0,2048) of mt.  Per 128-row chunk c, one PSUM bank:
            #   PE: pg[:,0:128] = Xc^T, pg[:,128:256] = Yc^T (identity mm)
            #   ACT: XTN[:,c] = -2*Xc^T (bf16), YT[:,c] = Yc^T (bf16),
            #        sqA/sqB = Square(transposes) (f32 SBUF)
            #   PE: pg[0,256:384] = ones^T @ sqA = |x_s|^2 row (same for y)
            #   ACT: xq[0, c*128:+128] / yq[1, ...] = norm rows (bf16)
            for c in range(NCH):
                r0 = 128 * c
                g0 = 512 * (c // 2) + (c % 2) * 128
                Xc = xcp.tile([128, 128], f8, tag="Xc")
                Yc = xcp.tile([128, 128], f8, tag="Yc")
                nc.sync.dma_start(Xc[:], gath_d[g0:g0 + 128, :])
                nc.sync.dma_start(Yc[:], gath_d[g0 + 256:g0 + 384, :])
                pg = gpsp.tile([128, 512], f32, tag="pg")
                nc.tensor.matmul(pg[:, 0:128], Xc[:], eye[:], start=True, stop=True)
                nc.tensor.matmul(pg[:, 128:256], Yc[:], eye[:], start=True, stop=True)
                nc.scalar.activation(
                    XTN[:, r0:r0 + 128], pg[:, 0:128], AF.Copy, scale=-2.0
                )
                nc.scalar.activation(YT[:, r0:r0 + 128], pg[:, 128:256], AF.Copy)
                sqA = sqp.tile([128, 128], f32, tag="sqA")
                sqB = sqp.tile([128, 128], f32, tag="sqB")
                nc.scalar.activation(sqA[:], pg[:, 0:128], AF.Square)
                nc.scalar.activation(sqB[:], pg[:, 128:256], AF.Square)
                nc.tensor.matmul(
                    pg[0:1, 256:384], ones128[:], sqA[:], start=True, stop=True
                )
                nc.tensor.matmul(
                    pg[0:1, 384:512], ones128[:], sqB[:], start=True, stop=True
                )
                nc.scalar.activation(xq[0:1, r0:r0 + 128], pg[0:1, 256:384], AF.Copy)
                yst = xcp.tile([1, 128], bf16, tag="yst")
                nc.scalar.activation(yst[:], pg[0:1, 384:512], AF.Copy)
                nc.gpsimd.dma_start(yq[1:2, r0:r0 + 128], yst[:])

            # ---------------- d production ------------------------------
            # block (ib, jc): DP rows [128ib, 128ib+128) x cols [512jc, +512)
            def emit_mm(pg, ib, jc, sl):
                i0, a = 128 * ib, 512 * jc + 128 * sl
                nc.tensor.matmul(
                    pg[:, 128 * sl:128 * sl + 128],
                    XTN[:, i0:i0 + 128], YT[:, a:a + 128],
                    start=True, stop=False,
                )
                nc.tensor.matmul(
                    pg[:, 128 * sl:128 * sl + 128],
                    xq[:, i0:i0 + 128], yq[:, a:a + 128],
                    start=False, stop=True,
                )

            def emit_sqrt(pg, st, sl):
                gap = bass.AP(
                    st.tensor, BW * 8 * sl + 1,
                    [[32 * BW, 128], [BW, 8], [1, W]],
                )
                nc.scalar.activation(gap, pg[:, 128 * sl:128 * sl + 128], AF.Sqrt)

            def emit_dma(st, ib, jc, h):
                # 8 per-chunk DMAs (BIR APs cannot step partitions off-pitch);
                # issued from the Pool queue whose DMA dispatch is ~20x
                # cheaper than SP's
                i0 = 128 * ib
                for cl in range(8 * h, 8 * h + 8):
                    c = 32 * jc + cl
                    src = st[:, cl * BW:(cl + 1) * BW]
                    dst = bass.AP(
                        dsk.tensor,
                        c * DSKC + (i0 + SL * c + SL) * BW,
                        [[DSKC, 1], [BW, 128], [1, BW]],
                    )
                    nc.gpsimd.dma_start(dst, src)

            def emit_block(ib, jc):
                pg = gpsp.tile([128, 512], f32, tag="pg")
                st = stg_tiles[0]
                stg_tiles.reverse()
                for sl in range(4):
                    emit_mm(pg, ib, jc, sl)
                for sl in range(4):
                    emit_sqrt(pg, st, sl)
                for h in range(4):
                    emit_dma(st, ib, jc, h)

            # first 3 i-blocks fully in the prologue (needed from step 0)
            for ib in range(3):
                for jc in range(4):
                    emit_block(ib, jc)

            # remaining 52 blocks paced into the wavefront, deadline order.
            # each block = 10 slots (4 mm-pairs, 4 sqrts, 2 dma) 3 steps apart
            rest = sorted(
                [(ib, jc) for ib in range(3, S // 128) for jc in range(4)],
                key=lambda b: 128 * b[0] + 96 * b[1],
            )
            sched = {}
            blk_state = {}
            for n, (ib, jc) in enumerate(rest):
                base = 10 + 32 * n
                for s in range(4):
                    sched.setdefault(base + 3 * s, []).append(("mm", ib, jc, s))
                for s in range(4):
                    sched.setdefault(base + 12 + 3 * s, []).append(("sq", ib, jc, s))
                for h in range(4):
                    sched.setdefault(base + 22 + 2 * h, []).append(("dm", ib, jc, h))

            # MAE chunk k: prefetch DMA at E_k, DVE consume at E_k + 220
            ddv = dd_d.rearrange("(p x) f -> p (x f)", p=128)
            mae_tiles = {}
            for k in range(n_mch):
                sched.setdefault(120 + 260 * k, []).append(("ml", k))
                sched.setdefault(340 + 260 * k, []).append(("mc", k))

            def emit_sched(t):
                for op in sched.pop(t, ()):
                    if op[0] == "mm":
                        _, ib, jc, sl = op
                        if sl == 0:
                            pg = gpsp.tile([128, 512], f32, tag="pg")
                            st = stg_tiles[0]
                            stg_tiles.reverse()
                            blk_state[(ib, jc)] = (pg, st)
                        pg, st = blk_state[(ib, jc)]
                        emit_mm(pg, ib, jc, sl)
                    elif op[0] == "sq":
                        _, ib, jc, sl = op
                        pg, st = blk_state[(ib, jc)]
                        emit_sqrt(pg, st, sl)
                    elif op[0] == "dm":
                        _, ib, jc, h = op
                        pg, st = blk_state[(ib, jc)]
                        emit_dma(st, ib, jc, h)
                    elif op[0] == "ml":
                        k = op[1]
                        ta = mchp.tile([128, MCH], f8, tag="ma")
                        nc.sync.dma_start(ta[:], ddv[:, k * MCH:(k + 1) * MCH])
                        mae_tiles[k] = ta
                    else:  # "mc"
                        k = op[1]
                        ta = mae_tiles.pop(k)
                        red = redp.tile([128, 1], f32, tag="red")
                        nc.vector.tensor_reduce(
                            red[:], ta[:], mybir.AxisListType.X, AL.add,
                            apply_absolute_value=True,
                        )
                        nc.vector.tensor_tensor(macc[:], macc[:], red[:], AL.add)

            # ---------------- wavefront ---------------------------------
            cur_ps = None
            for t in range(NSTEP):
                if t % 2 == 0:
                    # seeds for steps t, t+1: pst[c] = BIG@c0 + D-col16 of
                    # blocks (t-3),(t-2) shifted down one partition
                    cur_ps = spsp.tile([128, 2], f32, tag="pst")
                    nc.tensor.matmul(
                        cur_ps[:, 0:2], brow[:, 0:128], otwo[:],
                        start=True, stop=False,
                    )
                    nc.tensor.matmul(
                        cur_ps[:, 0:2], shT[:, 0:128],
                        dsk[:, BW * t + W:BW * t + W + 2 * BW:BW],
                        start=False, stop=True,
                    )
                emit_sched(t)
                o = BW * (t + SL)
                prev = dsk[:, o - BW:o]
                nc.vector.tensor_tensor(
                    mB[:, 1:BW], prev[:, 0:W], prev[:, 1:BW], AL.min
                )
                for _ in range(_SCAN_REPS):
                    nc.vector.tensor_tensor_scan(
                        dsk[:, o:o + BW], mB[:, 0:BW], dsk[:, o:o + BW],
                        cur_ps[:, (t % 2):(t % 2) + 1], AL.min, AL.add,
                    )
            # leftover sched events (none expected, but don't drop any)
            for t in sorted(list(sched.keys())):
                emit_sched(t)

            # ---------------- outputs -----------------------------------
            nc.sync.dma_start(out_d[0:1, 0:1], dsk[NB - 1:NB, DSKC - 1:DSKC])
            pm = gpsp.tile([128, 512], f32, tag="pg")
            nc.tensor.matmul(
                pm[0:1, 0:1], macc[:, 0:1], ones128[:, 0:1],
                start=True, stop=True,
            )
            nc.scalar.copy(mres[0:1, 0:1], pm[0:1, 0:1])
            nc.sync.dma_start(out_d[0:1, 1:2], mres[0:1, 0:1])

    return nc


# ---------------------------------------------------------------- runtime
_CACHE = {}


def _get_nc(S, F, mae_rows):
    key = (S, F, mae_rows)
    if key not in _CACHE:
        _CACHE[key] = _build_nc(S, F, mae_rows)
    return _CACHE[key]


class _Runner:
    """Runs nc on 8 cores.  First call goes through run_bass_kernel_spmd
    (compile + validate); later calls reuse a cached compiled executable of
    the identical shard_map computation, skipping the ~1s re-trace/re-lower
    that run_bass_kernel_spmd pays per call."""

    def __init__(self, nc, n_cores=8):
        import concourse.mybir as mybir

        self.nc = nc
        self.n_cores = n_cores
        self.compiled = None
        self.ran_spmd = False

        part = nc.partition_id_tensor.name if nc.partition_id_tensor else None
        self.partition_name = part
        in_names, out_names, out_shapes, out_dtypes = [], [], [], []
        for alloc in nc.m.functions[0].allocations:
            if not isinstance(alloc, mybir.MemoryLocationSet):
                continue
            name = alloc.memorylocations[0].name
            if alloc.kind == "ExternalInput":
                if name != part:
                    in_names.append(name)
            elif alloc.kind == "ExternalOutput":
                out_names.append(name)
                out_shapes.append(tuple(alloc.tensor_shape))
                out_dtypes.append(mybir.dt.np(alloc.dtype))
        self.in_names = in_names
        self.out_names = out_names
        self.out_shapes = out_shapes
        self.out_dtypes = out_dtypes

    def _build_compiled(self, concat_in):
        import jax
        import numpy as np
        from jax.sharding import Mesh, PartitionSpec
        from jax.experimental.shard_map import shard_map
        from concourse.bass2jax import _bass_exec_p, install_neuronx_cc_hook

        install_neuronx_cc_hook()
        nc, n_cores = self.nc, self.n_cores
        out_avals = tuple(
            jax.core.ShapedArray(s, d)
            for s, d in zip(self.out_shapes, self.out_dtypes)
        )
        all_names = list(self.in_names) + list(self.out_names)
        if self.partition_name is not None:
            all_names.append(self.partition_name)
        n_params, n_outs = len(self.in_names), len(self.out_names)
        out_names = tuple(self.out_names)
        partition_name = self.partition_name

        def _body(*args):
            operands = list(args)
            if partition_name is not None:
                from concourse.bass2jax import partition_id_tensor

                operands.append(partition_id_tensor())
            outs = _bass_exec_p.bind(
                *operands,
                out_avals=out_avals,
                in_names=tuple(all_names),
                out_names=out_names,
                lowering_input_output_aliases=(),
                sim_require_finite=True,
                sim_require_nnan=True,
                nc=nc,
            )
            return tuple(outs)

        devices = jax.devices()[:n_cores]
        mesh = Mesh(np.asarray(devices), ("core",))
        in_specs = (PartitionSpec("core"),) * (n_params + n_outs)
        out_specs = (PartitionSpec("core"),) * n_outs
        donate = tuple(range(n_params, n_params + n_outs))
        sharded = jax.jit(
            shard_map(
                _body, mesh=mesh, in_specs=in_specs, out_specs=out_specs,
                check_rep=False,
            ),
            donate_argnums=donate,
            keep_unused=True,
        )
        lowered = sharded.lower(*concat_in, *self._concat_zeros())
        self.compiled = lowered.compile()

    def _concat_zeros(self):
        return [
            np.zeros((self.n_cores * s[0], *s[1:]), d)
            for s, d in zip(self.out_shapes, self.out_dtypes)
        ]

    def _run_fast(self, concat_in):
        import jax
        from jax.sharding import Mesh, NamedSharding, PartitionSpec

        if self.compiled is None:
            self._build_compiled(concat_in)
        # explicit device_put is ~70ms cheaper than letting the executable
        # dispatch convert host ndarrays itself
        mesh = Mesh(np.asarray(jax.devices()[:self.n_cores]), ("core",))
        sh = NamedSharding(mesh, PartitionSpec("core"))
        dev_in = [jax.device_put(a, sh) for a in concat_in]
        out_arrs = self.compiled(*dev_in, *self._concat_zeros())
        jax.block_until_ready(out_arrs)
        return [
            {
                name: np.asarray(out_arrs[i]).reshape(
                    self.n_cores, *self.out_shapes[i]
                )[c]
                for i, name in enumerate(self.out_names)
            }
            for c in range(self.n_cores)
        ]

    def run(self, concat_in):
        """concat_in: arrays in self.in_names order, axis-0-concatenated
        over cores.  Returns per-core dicts of outputs."""
        if not self.ran_spmd:
            # first call: the documented compile+run path (also warms the
            # NEFF cache for the cached fast path, which reuses the same
            # backend compile).
            from concourse.bass_utils import run_bass_kernel_spmd

            shard0 = [a.shape[0] // self.n_cores for a in concat_in]
            in_maps = [
                {
                    name: concat_in[i][c * shard0[i]:(c + 1) * shard0[i]]
                    for i, name in enumerate(self.in_names)
                }
                for c in range(self.n_cores)
            ]
            res = run_bass_kernel_spmd(
                self.nc, in_maps, core_ids=list(range(self.n_cores))
            )
            self.ran_spmd = True
            return res.results
        try:
            return self._run_fast(concat_in)
        except Exception:
            from concourse.bass_utils import run_bass_kernel_spmd

            shard0 = [a.shape[0] // self.n_cores for a in concat_in]
            in_maps = [
                {
                    name: concat_in[i][c * shard0[i]:(c + 1) * shard0[i]]
                    for i, name in enumerate(self.in_names)
                }
                for c in range(self.n_cores)
            ]
            res = run_bass_kernel_spmd(
                self.nc, in_maps, core_ids=list(range(self.n_cores))
            )
            return res.results


def _get_runner(S, F, mae_rows):
    key = ("runner", S, F, mae_rows)
    if key not in _CACHE:
        _CACHE[key] = _Runner(_get_nc(S, F, mae_rows))
    return _CACHE[key]


_FP8_LUT = None


def _to_fp8_bits(x32):
    """f32 -> fp8e4m3 via RTNE-to-bf16 then a 64K LUT (2x faster than
    ml_dtypes' direct cast; the double rounding moves only exact-halfway
    cases by <=1 fp8 ulp)."""
    import ml_dtypes

    global _FP8_LUT
    if _FP8_LUT is None:
        _FP8_LUT = (
            np.arange(65536, dtype=np.uint16)
            .view(ml_dtypes.bfloat16)
            .astype(ml_dtypes.float8_e4m3)
            .view(np.uint8)
        )
    return _FP8_LUT[x32.astype(ml_dtypes.bfloat16).view(np.uint16)]


def _prepare(pred, target):
    """Host-side input prep: fp8 diff shards for the MAE, plus interleaved
    1/8 slices of (pred[0], target[0]) for the on-device AllGather."""
    import ml_dtypes

    B, S, F = pred.shape
    n_cores = 8
    shard = B // n_cores
    mae_rows = shard * S
    fp8 = ml_dtypes.float8_e4m3
    pred = np.ascontiguousarray(pred, dtype=np.float32)
    target = np.ascontiguousarray(target, dtype=np.float32)
    dd_all = _to_fp8_bits(pred - target).reshape(B * S, F).view(fp8)
    xs = S // n_cores
    xy = np.empty((n_cores, 2 * xs, F), np.uint8)
    xy[:, :xs] = _to_fp8_bits(pred[0]).reshape(n_cores, xs, F)
    xy[:, xs:] = _to_fp8_bits(target[0]).reshape(n_cores, xs, F)
    xy_all = xy.reshape(n_cores * 2 * xs, F).view(fp8)
    return [dd_all, xy_all], mae_rows


def kernel(pred, target):
    pred = np.asarray(pred)
    target = np.asarray(target)
    B, S, F = pred.shape
    concat_in, mae_rows = _prepare(pred, target)
    runner = _get_runner(S, F, mae_rows)
    results = runner.run(concat_in)
    mae_sum = sum(float(results[k]["out"][0, 1]) for k in range(8))
    dtw = float(results[0]["out"][0, 0])
    loss = 0.5 * (mae_sum / (B * S * F)) + 0.5 * (dtw / (S * F))
    return np.float32(loss)
